# revision 82
# baseline (speedup 1.0000x reference)
"""Trainium2 Bass kernel v3 for a dense transformer block (B=4, T=2048, D=1024, H=16).

Sharding: 8 cores = 4 batches x 2 query-parity groups (zigzag), slots
processed in mixed order ORDER = [7,0,1,6,2,3,5,4] (NK blocks
[2,16,14,4,12,10,6,8]) so that:
  - K/V block-pair pools release progressively (big slots early) while FF1
    state grows -- complementary SBUF footprints;
  - exp (ACT-bound) of big slots overlaps FF1/proj/tails (PE-dense fillers).

Structure per core:
  prefix:   x DMA -> LN1 -> transpose (pipelined per 4-block group; the first
            x tiles lead the weight streams), V/K/Q production interleaved.
  attn:     per position: scores (head-paired K=64, row-tiled) -> exp(ACT) ->
            mask(Pool) -> PV (ones-column denominators) -> scale -> att;
            tail: transpose, proj, + bf16 residual (proj bias folded host-
            side) -> x2, LN2 -> XT2. xq residual tiles prefetched one tail
            ahead. FF1 interleaved as PE filler; gelu -> stage -> HT to DRAM.
  ff2:      streams HT (prefetched one hhg ahead) + W2 (10-deep ring) from
            DRAM, 8-bank PSUM accumulate; ff2 bias folded into x2 reload on
            Pool so the tail is a single DVE add per tile.
Queue isolation (x/xq/weights on ACT, masks on Pool, HT/W2/y on SP) lets the
next iteration's LN1 prefix overlap this iteration's FF2 phase.
"""

import sys

try:
    import concourse  # noqa: F401
except ImportError:
    sys.path.insert(0, "/opt/trn_rl_repo")

import numpy as np
import ml_dtypes

import concourse.bass as bass
import concourse.bacc as bacc
import concourse.tile as tile
from concourse.tile import add_dep_helper
from concourse import mybir
from concourse.bass_utils import run_bass_kernel_spmd

F32 = mybir.dt.float32
BF16 = mybir.dt.bfloat16
AF = mybir.ActivationFunctionType
ALU = mybir.AluOpType
PSUM = bass.MemorySpace.PSUM

B, T, D = 4, 2048, 1024
H, HD = 16, 64
HID = 4 * D
EPS = 1e-5
N_CORES = 8
NB = T // 128  # 16 token blocks per batch
NQ = 8  # query slots per core
TQ = NQ * 128  # query tokens per core

# zigzag query-block assignment (parity balances causal work)
QB = [
    [15, 12, 11, 8, 7, 4, 3, 0],
    [14, 13, 10, 9, 6, 5, 2, 1],
]
NKMAX = [16, 14, 12, 10, 8, 6, 4, 2]
# processing order: position -> zigzag slot index. 2-blk slot first (its Q/K
# deferral fillers run under it), then big slots early so kv pools release
# while FF1 state grows.
ORDER = [7, 0, 1, 6, 2, 3, 5, 4]
NK = [NKMAX[o] for o in ORDER]  # [2, 16, 14, 4, 12, 10, 6, 8]


def _perm(p):
    idx = []
    for pos in range(NQ):
        qb = QB[p][ORDER[pos]]
        idx.extend(range(qb * 128, (qb + 1) * 128))
    return np.array(idx, dtype=np.int64)


def _masks(p):
    """[NQ, 128, 256] multiplicative mask for the last two key blocks per pos."""
    m = np.zeros((NQ, 128, 256), dtype=np.float32)
    tri = np.triu(np.ones((128, 128), np.float32))  # valid where k_local <= q_local
    for pos in range(NQ):
        nk = QB[p][ORDER[pos]] + 1
        nkm = NK[pos]
        for c in range(2):
            ki = nkm - 2 + c
            if ki < nk - 1:
                m[pos, :, c * 128:(c + 1) * 128] = 1.0
            elif ki == nk - 1:
                m[pos, :, c * 128:(c + 1) * 128] = tri
    return m


# ---------------------------------------------------------------------------
# Device program
# ---------------------------------------------------------------------------

def _layernorm_tile(nc, pool, x_ap, out_ap, eps_t, norm_engine=None):
    """LN (without gain/bias) of a [128, D] tile -> out_ap bf16."""
    stats = pool.tile([128, 2, nc.vector.BN_STATS_DIM], F32, tag="ln_stats")
    mv = pool.tile([128, nc.vector.BN_AGGR_DIM], F32, tag="ln_mv")
    nmr = pool.tile([128, 1], F32, tag="ln_nmr")
    xg = x_ap.rearrange("p (s d) -> p s d", s=2)
    for s in range(2):
        nc.vector.bn_stats(out=stats[:, s, :], in_=xg[:, s, :])
    nc.vector.bn_aggr(out=mv[:], in_=stats[:])
    mean = mv[:, 0:1]
    rstd = mv[:, 1:2]
    nc.scalar.activation(out=rstd, in_=rstd, func=AF.Sqrt, bias=eps_t[:], scale=1.0)
    nc.vector.reciprocal(out=rstd, in_=rstd)
    nc.vector.tensor_scalar(
        out=nmr[:], in0=mean, scalar1=rstd, scalar2=-1.0,
        op0=ALU.mult, op1=ALU.mult,
    )
    if norm_engine == "vector":
        nc.vector.tensor_scalar(
            out=out_ap, in0=x_ap, scalar1=rstd, scalar2=nmr[:],
            op0=ALU.mult, op1=ALU.add,
        )
    else:
        nc.scalar.activation(out=out_ap, in_=x_ap, func=AF.Identity,
                             bias=nmr[:], scale=rstd)


def build_program(niter=None, stop_after=None):
    nc = bacc.Bacc("TRN2", target_bir_lowering=False, debug=False,
                   num_devices=N_CORES)

    xf_d = nc.dram_tensor("x_full", [T, D], BF16, kind="ExternalInput")
    xq_d = nc.dram_tensor("x_q", [TQ, D], BF16, kind="ExternalInput")
    xqr_d = nc.dram_tensor("x_qr", [TQ, D], BF16, kind="ExternalInput")
    mask_d = nc.dram_tensor("mask", [128, NQ, 256], BF16, kind="ExternalInput")
    wqkv_d = nc.dram_tensor("wqkv", [128, 8, 3 * D], BF16, kind="ExternalInput")
    wproj_d = nc.dram_tensor("wproj", [128, 8, D], BF16, kind="ExternalInput")
    wff1_d = nc.dram_tensor("wff1", [128, 8, HID], BF16, kind="ExternalInput")
    wff2_d = nc.dram_tensor("wff2", [128, 32, D], BF16, kind="ExternalInput")
    bqk_d = nc.dram_tensor("bqk", [128, 16], F32, kind="ExternalInput")
    bv_d = nc.dram_tensor("bv", [D], F32, kind="ExternalInput")
    bff1_d = nc.dram_tensor("bff1", [128, 32], F32, kind="ExternalInput")
    bff2b_d = nc.dram_tensor("bff2b", [D], BF16, kind="ExternalInput")
    y_d = nc.dram_tensor("y", [TQ, D], F32, kind="ExternalOutput")
    ht_d = nc.dram_tensor("ht_scratch", [128, 4, 32, 256], BF16, kind="Internal")
    x2_d = nc.dram_tensor("x2_scratch", [TQ, D], BF16, kind="Internal")

    def bcast_row(dram_handle, n):
        ap = dram_handle.ap()
        return bass.AP(tensor=ap.tensor, offset=ap.offset, ap=[[0, 128], [1, n]])

    dram = dict(xf=xf_d, xq=xq_d, xqr=xqr_d, wqkv=wqkv_d, wproj=wproj_d,
                wff1=wff1_d, wff2=wff2_d, y=y_d, ht=ht_d, x2=x2_d)

    with tile.TileContext(nc) as tc:
        with tc.tile_pool(name="singles", bufs=1) as singles:
            ident = singles.tile([128, 128], BF16)
            from concourse.masks import make_identity
            make_identity(nc, ident[:])
            nc._ident = ident

            st = {}
            st["eps"] = singles.tile([128, 1], F32, name="eps", tag="eps")
            nc.vector.memset(st["eps"][:], EPS)
            for nm, dt, shape, src in (
                ("bqk", F32, [128, 16], bqk_d), ("bff1", F32, [128, 32], bff1_d),
                ("mask", BF16, [128, NQ, 256], mask_d),
            ):
                st[nm] = singles.tile(shape, dt, name=nm, tag=nm)
                nc.sync.dma_start(st[nm][:], src.ap())
            st["fb2"] = singles.tile([128, D], BF16, name="fb2", tag="fb2")
            nc.gpsimd.dma_start(st["fb2"][:], bcast_row(bff2b_d, D))
            nc._bcast_row = bcast_row
            nc._bv_d = bv_d

            if niter is None:
                _block_body(tc, nc, dram, st, stop_after)
            elif isinstance(niter, str) and niter.startswith("unroll"):
                for _ in range(int(niter[6:])):
                    _block_body(tc, nc, dram, st, stop_after)
            else:
                with tc.For_i(0, niter, 1):
                    _block_body(tc, nc, dram, st, stop_after)

    nc.compile()
    return nc


def _block_body(tc, nc, dram, st, stop_after=None):
    xf_d, xq_d, y_d = dram["xf"], dram["xq"], dram["y"]
    ht_d, x2_d = dram["ht"], dram["x2"]
    eps_t = st["eps"]

    # ---- persistent pools --------------------------------------------------
    # right side (bottom->top): qt, kv[0..7], xtq, xtf
    qt_p = tc.alloc_tile_pool(name="qt", bufs=1, side="right")
    QT = qt_p.tile([128, 8, TQ], BF16, name="QT", tag="qt")
    kv = []
    kvt = []
    for b in range(8):
        p = tc.alloc_tile_pool(name=f"kv{b}", bufs=1, side="right")
        KT2 = p.tile([128, 8, 256], BF16, name=f"KT{b}", tag="kt")
        V2 = p.tile([128, 2, H, HD + 1], BF16, name=f"V{b}", tag="v")
        nc.vector.memset(V2[:, :, :, HD:HD + 1], 1.0)
        kv.append(p)
        kvt.append((KT2, V2))
    xtq_p = tc.alloc_tile_pool(name="xtq", bufs=1, side="right")
    XTQ = xtq_p.tile([128, 8, TQ], BF16, name="XTQ", tag="xtq")
    xtf_p = tc.alloc_tile_pool(name="xtf", bufs=1, side="right")
    XTF = xtf_p.tile([128, 8, T], BF16, name="XTF", tag="xtf")

    def KTb(kk, po, hc):  # [64, 128] stationary view of key block kk
        return kvt[kk // 2][0][po:po + 64, hc, (kk % 2) * 128:(kk % 2 + 1) * 128]

    def Vb(kk, h):  # [128, HD+1] view of key block kk, head h
        return kvt[kk // 2][1][:, kk % 2, h, :]

    pw_p = tc.alloc_tile_pool(name="wproj", bufs=1, side="left")
    PW = pw_p.tile([128, 8, D], BF16, name="PW", tag="pw")

    # ---- prefix: fused LN1 + transpose + V/K/Q production ------------------
    wqk_p = tc.alloc_tile_pool(name="wqk", bufs=3, side="left")
    p1 = tc.alloc_tile_pool(name="p1", bufs=6, side="left")
    p1s = tc.alloc_tile_pool(name="p1s", bufs=8, side="left")
    xln_p = tc.alloc_tile_pool(name="xln", bufs=6, side="left")
    wv_p = tc.alloc_tile_pool(name="wv", bufs=1, side="left")
    pre_tp = tc.alloc_tile_pool(name="pre_tp", bufs=2, space=PSUM)
    pre_mm = tc.alloc_tile_pool(name="pre_mm", bufs=2, space=PSUM)

    def ln_group(x_d, t0, n, XT):
        """LN + transpose n token blocks starting at t0 from x_d into XT."""
        tiles = []
        for u in range(n):
            t = t0 + u
            xt = p1.tile([128, D], BF16, tag="xt", name="xt")
            q = nc.gpsimd if (x_d is xf_d and u % 2 == 0) else nc.scalar
            q.dma_start(xt[:], x_d.ap()[t * 128:(t + 1) * 128, :])
            ln = xln_p.tile([128, D], BF16, tag="ln", name="ln")
            _layernorm_tile(nc, p1s, xt[:], ln[:], eps_t)
            tiles.append(ln)
        for dc in range(8):
            pt = pre_tp.tile([128, n * 128], BF16, tag="tp", name="pt",
                             padded_shape=[128, 1024])
            for u in range(n):
                nc.tensor.transpose(
                    pt[:, u * 128:(u + 1) * 128],
                    tiles[u][:, dc * 128:(dc + 1) * 128], nc._ident[:])
            dst = XT[:, dc, t0 * 128:(t0 + n) * 128]
            if dc % 2 == 1:
                nc.scalar.copy(out=dst, in_=pt[:])
            else:
                nc.vector.tensor_copy(dst, pt[:])

    def emit_v(tg, wv_tiles, vb_t):
        """V for token blocks 4tg..4tg+3 (needs XTF of those blocks)."""
        for vh in range(2):
            for u in range(4):
                t = tg * 4 + u
                ps = pre_mm.tile([128, 512], F32, tag="mm", name="ps")
                for dc in range(8):
                    nc.tensor.matmul(
                        ps[:], XTF[:, dc, t * 128:(t + 1) * 128],
                        wv_tiles[vh][:, dc, :], start=(dc == 0), stop=(dc == 7),
                    )
                nc.vector.tensor_add(
                    out=kvt[t // 2][1][:, t % 2, vh * 8:(vh + 1) * 8, 0:HD],
                    in0=ps[:].rearrange("p (h d) -> p h d", h=8),
                    in1=vb_t[:, vh * 512:(vh + 1) * 512]
                        .rearrange("p (h d) -> p h d", h=8),
                )

    def emit_k(cc, pairs):
        """KT chunk occ=cc-8 for token pair(s) — one weight fetch shared
        across all pairs whose XTF is ready."""
        occ = cc - 8
        w = wqk_p.tile([128, 8, 128], BF16, tag="wqk", name="w")
        q = nc.sync if cc % 2 == 0 else nc.gpsimd
        q.dma_start(
            w[:], dram["wqkv"].ap()[:, :, cc * 128:(cc + 1) * 128])
        for pair in pairs:
            pshs = [pre_mm.tile([128, 512], F32, tag="mm", name="ps")
                    for _ in range(2)]
            for h2 in range(2):
                for dc in range(8):
                    nc.tensor.matmul(
                        pshs[h2][:],
                        w[:, dc, :],
                        XTF[:, dc, (pair * 2 + h2) * 512:
                            (pair * 2 + h2 + 1) * 512],
                        start=(dc == 0), stop=(dc == 7),
                    )
            for g in range(4):
                b = pair * 4 + g
                nc.vector.tensor_scalar_add(
                    out=kvt[b][0][:, occ, :],
                    in0=pshs[g // 2][:, (g % 2) * 256:(g % 2 + 1) * 256],
                    scalar1=st["bqk"][:, cc:cc + 1],
                )

    def emit_q_prefix(cc):
        """Q chunk cc for positions 0-1 (cols 0-255) only."""
        w = wqk_p.tile([128, 8, 128], BF16, tag="wqk", name="w")
        q = nc.sync if cc % 2 == 0 else nc.gpsimd
        q.dma_start(
            w[:], dram["wqkv"].ap()[:, :, cc * 128:(cc + 1) * 128])
        ps = pre_mm.tile([128, 256], F32, tag="mm", name="ps",
                         padded_shape=[128, 512])
        for dc in range(8):
            nc.tensor.matmul(
                ps[:], w[:, dc, :], XTQ[:, dc, 0:256],
                start=(dc == 0), stop=(dc == 7),
            )
        nc.vector.tensor_scalar_add(
            out=QT[:, cc, 0:256], in0=ps[:], scalar1=st["bqk"][:, cc:cc + 1],
        )

    # pipelined prefix: x DMAs lead (weight streams issued after the first
    # LN groups so the first tiles aren't queued behind bulk weights)
    wv_tiles = [wv_p.tile([128, 8, 512], BF16, tag=f"wv{vh}", name=f"wv{vh}")
                for vh in range(2)]
    vb_t = wv_p.tile([128, D], F32, name="vb", tag="vb")

    ln_group(xf_d, 0, 2, XTF)
    for vh in range(2):
        nc.sync.dma_start(
            wv_tiles[vh][:], dram["wqkv"].ap()[:, :, 2 * D + vh * 512:
                                               2 * D + (vh + 1) * 512])
    nc.gpsimd.dma_start(vb_t[:], nc._bcast_row(nc._bv_d, D))
    ln_group(xf_d, 2, 2, XTF)
    ln_group(xf_d, 4, 4, XTF)
    emit_v(0, wv_tiles, vb_t)
    ln_group(xf_d, 8, 4, XTF)
    emit_v(1, wv_tiles, vb_t)
    for cc in range(8, 12):
        emit_k(cc, (0,))
    ln_group(xf_d, 12, 4, XTF)
    for cc in range(12, 16):
        emit_k(cc, (0, 1))
    emit_v(2, wv_tiles, vb_t)
    ln_group(xq_d, 0, 4, XTQ)
    emit_v(3, wv_tiles, vb_t)
    for cc in range(8, 12):
        emit_k(cc, (1,))
    ln_group(xq_d, 4, 4, XTQ)
    for cc in range(8):
        emit_q_prefix(cc)

    pre_mm.release()
    pre_tp.release()
    wv_p.release()
    xln_p.release()
    p1s.release()
    p1.release()
    xtf_p.release()
    if stop_after == "prefix":
        xtq_p.release()
        wqk_p.release()
        for b in reversed(range(8)):
            kv[b].release()
        qt_p.release()
        pw_p.release()
        return

    # ---- attention + interleaved FF1 --------------------------------------
    xt2_p = tc.alloc_tile_pool(name="xt2", bufs=1, side="left")
    XT2P = [xt2_p.tile([128, 8, 256], BF16, name=f"XT2_{q}", tag=f"xt2{q}")
            for q in range(4)]

    sc_ps = tc.alloc_tile_pool(name="sc_ps", bufs=2, space=PSUM)
    pv_ps = tc.alloc_tile_pool(name="pv_ps", bufs=1, space=PSUM)
    tp_ps = tc.alloc_tile_pool(name="tp_ps", bufs=1, space=PSUM)
    pj_ps = tc.alloc_tile_pool(name="pj_ps", bufs=1, space=PSUM)
    f1_ps = tc.alloc_tile_pool(name="f1_ps", bufs=1, space=PSUM)

    att_pool = tc.alloc_tile_pool(name="attsl", bufs=2, side="left")
    attt_pool = tc.alloc_tile_pool(name="atttsl", bufs=1, side="left")
    atp = tc.alloc_tile_pool(name="at", bufs=5, side="left")
    epp = tc.alloc_tile_pool(name="ep", bufs=2, side="left")
    p7 = tc.alloc_tile_pool(name="p7", bufs=1, side="left")
    x2t_pool = tc.alloc_tile_pool(name="x2t", bufs=2, side="left")
    hst_p = tc.alloc_tile_pool(name="hstage", bufs=1, side="left")
    w1_holder = {}

    def emit_scores_pair(hc, pos, att_j):
        """Scores+exp+mask for BOTH heads of pair hc at position pos."""
        nblk = NK[pos]
        out = []
        for hp in range(2):
            out.append((2 * hc + hp, pos, [], att_j))
        for g in range((nblk + 7) // 8):
            blo = g * 8
            bhi = min(blo + 8, nblk)
            ncol = (bhi - blo) * 128
            pss = [sc_ps.tile([128, 1024], F32, tag="sc", name="ps")
                   for _ in range(2)]
            for kk in range(blo, bhi):
                for hp in range(2):
                    po = hp * 64
                    nc.tensor.matmul(
                        pss[hp][:, (kk - blo) * 128:(kk - blo + 1) * 128],
                        KTb(kk, po, hc),
                        QT[po:po + 64, hc, pos * 128:(pos + 1) * 128],
                        start=True, stop=True,
                    )
            for hp in range(2):
                at = atp.tile([128, 1024], BF16, tag="at", name="at")
                nc.scalar.activation(out=at[:, 0:ncol], in_=pss[hp][:, 0:ncol],
                                     func=AF.Exp, scale=0.125)
                for kk in range(max(blo, nblk - 2), bhi):
                    mc = (kk - (nblk - 2)) * 128
                    nc.gpsimd.tensor_mul(
                        out=at[:, (kk - blo) * 128:(kk - blo + 1) * 128],
                        in0=at[:, (kk - blo) * 128:(kk - blo + 1) * 128],
                        in1=st["mask"][:, pos, mc:mc + 128],
                    )
                out[hp][2].append((blo, bhi, at))
        return out

    def emit_pv_pair(pend2):
        """PV for both heads of a pair into one [128, 2, HD+1] PSUM tile."""
        pv = pv_ps.tile([128, 2, HD + 1], F32, tag="pv", name="pv")
        for hp, pend in enumerate(pend2):
            h, pos, ats, att_j = pend
            nblk = NK[pos]
            for blo, bhi, at in ats:
                for kk in range(blo, bhi):
                    nc.tensor.matmul(
                        pv[:, hp, :],
                        at[:, (kk - blo) * 128:(kk - blo + 1) * 128],
                        Vb(kk, h),
                        start=(kk == 0), stop=(kk == nblk - 1),
                    )
        for hp, pend in enumerate(pend2):
            h, pos, ats, att_j = pend
            r = epp.tile([128, 1], F32, tag="recip", name="r")
            nc.vector.reciprocal(out=r[:], in_=pv[:, hp, HD:HD + 1])
            nc.vector.tensor_scalar_mul(
                out=att_j[:, h * HD:(h + 1) * HD],
                in0=pv[:, hp, 0:HD], scalar1=r[:],
            )

    xq_pre = {}

    def fetch_xq(pos):
        t = p7.tile([128, D], BF16, tag="xq", name="xq", bufs=2)
        nc.sync.dma_start(t[:], dram["xqr"].ap()[pos * 128:(pos + 1) * 128, :])
        return t

    def slot_tail(pos, att_j):
        """att -> transpose -> proj -> +bias +resid -> x2t; LN2 -> XT2; spill."""
        attt = attt_pool.tile([128, 8, 128], BF16, tag="attt", name="attt")
        for g2 in range(2):
            pt = tp_ps.tile([128, 512], BF16, tag="tp", name="pt",
                            padded_shape=[128, 1024])
            for u in range(4):
                dc = g2 * 4 + u
                nc.tensor.transpose(
                    pt[:, u * 128:(u + 1) * 128],
                    att_j[:, dc * 128:(dc + 1) * 128], nc._ident[:])
            nc.vector.tensor_copy(attt[:, g2 * 4:(g2 + 1) * 4, :], pt[:])
        xq = xq_pre.pop(pos, None)
        if xq is None:
            xq = fetch_xq(pos)
        if pos + 1 < NQ and pos + 1 not in xq_pre:
            xq_pre[pos + 1] = fetch_xq(pos + 1)
        x2t = x2t_pool.tile([128, D], BF16, tag="x2t", name="x2t")
        for half in range(2):
            ps = pj_ps.tile([128, 512], F32, tag="pj", name="ps")
            for hcc in range(8):
                nc.tensor.matmul(
                    ps[:],
                    attt[:, hcc, :],
                    PW[:, hcc, half * 512:(half + 1) * 512],
                    start=(hcc == 0), stop=(hcc == 7),
                )
            nc.vector.tensor_add(
                out=x2t[:, half * 512:(half + 1) * 512],
                in0=ps[:], in1=xq[:, half * 512:(half + 1) * 512])
        x2_out_insts[pos] = nc.sync.dma_start(
            x2_d.ap()[pos * 128:(pos + 1) * 128, :], x2t[:])
        # LN2 on x2t -> bf16, then transpose into the position's XT2 pair tile
        ln2 = p7.tile([128, D], BF16, tag="ln2", name="ln2")
        _layernorm_tile(nc, epp, x2t[:], ln2[:], eps_t, norm_engine="vector")
        pr, qcol = pos // 2, (pos % 2) * 128
        for g2 in range(2):
            pt = tp_ps.tile([128, 512], BF16, tag="tp", name="pt",
                            padded_shape=[128, 1024])
            for u in range(4):
                dc = g2 * 4 + u
                nc.tensor.transpose(
                    pt[:, u * 128:(u + 1) * 128],
                    ln2[:, dc * 128:(dc + 1) * 128], nc._ident[:])
            nc.vector.tensor_copy(
                XT2P[pr][:, g2 * 4:(g2 + 1) * 4, qcol:qcol + 128],
                pt[:].rearrange("p (a b) -> p a b", a=4))

    def w_fetch(cc):
        w = wqk_p.tile([128, 8, 128], BF16, tag="wqk", name="w")
        nc.sync.dma_start(
            w[:], dram["wqkv"].ap()[:, :, cc * 128:(cc + 1) * 128])
        return w

    def q_filler(cc, w):
        """Deferred Q production: chunk cc, column blocks 1-3 (768 cols)."""
        for cb in range(1, 4):
            ps = f1_ps.tile([128, 2, 256], F32, tag="f1", name="ps")
            for dc in range(8):
                nc.tensor.matmul(
                    ps[:, 0, :], w[:, dc, :],
                    XTQ[:, dc, cb * 256:(cb + 1) * 256],
                    start=(dc == 0), stop=(dc == 7),
                )
            nc.vector.tensor_scalar_add(
                out=QT[:, cc, cb * 256:(cb + 1) * 256], in0=ps[:, 0, :],
                scalar1=st["bqk"][:, cc:cc + 1],
            )

    def ff1_chunk(pair, hhg):
        """FF1 hidden group hhg (8 hh) for position pair (256 tokens)."""
        if hhg < 2:
            W1, hoff = w1_holder["a"], 0
        elif hhg == 2:
            W1, hoff = w1_holder["b1"], 16
        else:
            W1, hoff = w1_holder["b2"], 24
        stage = hst_p.tile([128, 8, 256], BF16, tag="hs", name="hs")
        for hhi in range(0, 8, 2):
            ps = f1_ps.tile([128, 2, 256], F32, tag="f1", name="ps")
            for c in range(2):
                hh = hhg * 8 + hhi + c
                for dc in range(8):
                    nc.tensor.matmul(
                        ps[:, c, :],
                        W1[:, dc, (hh - hoff) * 128:(hh - hoff + 1) * 128],
                        XT2P[pair][:, dc, :],
                        start=(dc == 0), stop=(dc == 7),
                    )
            if hhi % 4 == 0 or pair == 3:
                nc.vector.tensor_copy(stage[:, hhi:hhi + 2, :], ps[:])
            else:
                nc.scalar.copy(out=stage[:, hhi:hhi + 2, :], in_=ps[:])
        ht_out_insts[(pair, hhg)] = nc.sync.dma_start(
            ht_d.ap()[:, pair, hhg * 8:(hhg + 1) * 8, :], stage[:])

    ht_out_insts = {}
    x2_out_insts = {}

    # filler queue: (weight_cc_or_None, compute) pairs, ~3-7us of PE work
    # each; weight DMAs prefetched 3 fillers ahead via the wqk ring.
    fillers = []
    for i in range(8):
        fillers.append((i, lambda w, c=i: q_filler(c, w)))
    prefetched = []

    def _prime():
        while len(prefetched) < 3 and fillers:
            cc2, fn2 = fillers.pop(0)
            prefetched.append((fn2, w_fetch(cc2) if cc2 is not None else None))

    _prime()

    def drain_filler(k=1):
        for _ in range(k):
            _prime()
            if not prefetched:
                return
            fn, w = prefetched.pop(0)
            fn(w)

    pending = None
    done = []
    wload_sched = {}
    for qtr in range(4):
        wload_sched[(0, qtr)] = (
            lambda q=qtr: nc.sync.dma_start(
                PW[:, 2 * q:2 * q + 2, :],
                dram["wproj"].ap()[:, 2 * q:2 * q + 2, :]))
    for pos in range(NQ):
        att_j = att_pool.tile([128, D], BF16, tag="att", name="att_j")
        for hc in range(8):
            if (pos, hc) in wload_sched:
                wload_sched.pop((pos, hc))()
            cur2 = emit_scores_pair(hc, pos, att_j)
            if pending is not None:
                emit_pv_pair(pending)
            pending = cur2
            if hc == 0 and done:
                slot_tail(*done.pop())
            elif pos < 2:
                drain_filler(2)
            elif pos < 4:
                if hc in (2, 4, 6):
                    drain_filler(2 if len(fillers) + len(prefetched) > 1 else 1)
            else:
                drain_filler(1)
        done.append((pos, att_j))
        if pos == 0:
            assert not fillers and not prefetched  # deferred Q done
            xtq_p.release()
            # W1 piece A (hh 0-15): allocate now (reusing XTQ's bytes) but
            # stream the halves mid-pos1 on sync, clear of the pos0 tail's
            # xq/x2 DMAs and of Pool's mask path.
            w1a_p = tc.alloc_tile_pool(name="w1a", bufs=1, side="left")
            W1A = w1a_p.tile([128, 8, HID // 2], BF16, name="W1A", tag="w1a")
            wload_sched[(1, 2)] = lambda: nc.sync.dma_start(
                W1A[:, :, 0:HID // 4], dram["wff1"].ap()[:, :, 0:HID // 4])
            wload_sched[(1, 5)] = lambda: nc.sync.dma_start(
                W1A[:, :, HID // 4:HID // 2],
                dram["wff1"].ap()[:, :, HID // 4:HID // 2])
            w1_holder["a"] = W1A
            w1_holder["pa"] = w1a_p
        if pos == 1:
            kv[7].release()
            # W1 piece B1 (hh 16-23), streamed mid-pos2
            w1b1_p = tc.alloc_tile_pool(name="w1b1", bufs=1, side="left")
            W1B1 = w1b1_p.tile([128, 8, HID // 4], BF16, name="W1B1",
                               tag="w1b1")
            wload_sched[(2, 3)] = lambda: nc.sync.dma_start(
                W1B1[:], dram["wff1"].ap()[:, :, HID // 2:3 * HID // 4])
            w1_holder["b1"] = W1B1
            w1_holder["pb1"] = w1b1_p
        if pos == 2:
            kv[6].release()
            # W1 piece B2 (hh 24-31), streamed mid-pos3
            w1b2_p = tc.alloc_tile_pool(name="w1b2", bufs=1, side="left")
            W1B2 = w1b2_p.tile([128, 8, HID // 4], BF16, name="W1B2",
                               tag="w1b2")
            wload_sched[(3, 3)] = lambda: nc.sync.dma_start(
                W1B2[:], dram["wff1"].ap()[:, :, 3 * HID // 4:])
            w1_holder["b2"] = W1B2
            w1_holder["pb2"] = w1b2_p
            for hhg in (0, 1):
                fillers.append((None, lambda w, h_=hhg: ff1_chunk(0, h_)))
        if pos == 3:
            for hhg in (2, 3):
                fillers.append((None, lambda w, h_=hhg: ff1_chunk(0, h_)))
        if pos == 4:
            for hhg in range(4):
                fillers.append((None, lambda w, h_=hhg: ff1_chunk(1, h_)))
        if pos == 6:
            for hhg in range(4):
                fillers.append((None, lambda w, h_=hhg: ff1_chunk(2, h_)))
    emit_pv_pair(pending)
    slot_tail(*done.pop())
    while fillers or prefetched:
        drain_filler()

    for b in (5, 4, 3, 2, 1, 0):
        kv[b].release()
    qt_p.release()

    # FF2 pools + tg0 prefetch issued BEFORE the last FF1 pair so the sync
    # queue has tg0's streams in flight while pair-3 FF1 finishes on PE.
    w2p = tc.alloc_tile_pool(name="w2", bufs=10, side="left")
    htp = tc.alloc_tile_pool(name="htin", bufs=2, side="left")
    htgp = tc.alloc_tile_pool(name="htgel", bufs=2, side="left")
    x2ip = tc.alloc_tile_pool(name="x2in", bufs=4, side="left")
    yp = tc.alloc_tile_pool(name="yp", bufs=2, side="left")

    pre = {}

    def x2_tile(pos):
        x2i = x2ip.tile([128, D], BF16, tag="x2i", name="x2i")
        rd = nc.sync.dma_start(x2i[:], x2_d.ap()[pos * 128:(pos + 1) * 128, :])
        add_dep_helper(rd.ins, x2_out_insts[pos].ins, True,
                       "x2 scratch RAW across DMA queues")
        # fold the ff2 bias in on Pool (idle during FF2) so the tail is one add
        nc.gpsimd.tensor_add(out=x2i[:], in0=x2i[:], in1=st["fb2"][:])
        return x2i

    def htt_tile(tg, hhg):
        htt = htp.tile([128, 2, 8, 256], BF16, tag="ht", name="htt")
        rd = nc.sync.dma_start(
            htt[:], ht_d.ap()[:, 2 * tg:2 * tg + 2, hhg * 8:(hhg + 1) * 8, :])
        for pr in (2 * tg, 2 * tg + 1):
            add_dep_helper(rd.ins, ht_out_insts[(pr, hhg)].ins, True,
                           "ht scratch RAW across DMA queues")
        return htt

    def w2_tile(hh):
        w2 = w2p.tile([128, D], BF16, tag="w2")
        nc.sync.dma_start(w2[:], dram["wff2"].ap()[:, hh, :])
        return w2

    pre["x2"] = [x2_tile(pos) for pos in range(4)]
    pre["htt"] = {(0, 0): htt_tile(0, 0)}
    pre["w2"] = [w2_tile(hh) for hh in range(3)]

    for hhg in range(4):
        ff1_chunk(3, hhg)

    f1_ps.release()
    pj_ps.release()
    tp_ps.release()
    pv_ps.release()
    sc_ps.release()
    ff2ps = tc.alloc_tile_pool(name="ff2ps", bufs=4, space=PSUM)

    for tg in range(2):
        x2in = pre.pop("x2") if tg == 0 else [x2_tile(tg * 4 + tt)
                                              for tt in range(4)]
        pss = [ff2ps.tile([128, 1024], F32, name="ym", tag="ym")
               for _ in range(4)]
        for hhg in range(4):
            htt = pre["htt"].pop((tg, hhg), None)
            if htt is None:
                htt = htt_tile(tg, hhg)
            htg = htgp.tile([128, 2, 8, 256], BF16, tag="htg", name="htg")
            for hhi in range(8):
                hh = hhg * 8 + hhi
                nc.scalar.activation(
                    out=htg[:, :, hhi, :], in_=htt[:, :, hhi, :],
                    func=AF.Gelu, bias=st["bff1"][:, hh:hh + 1], scale=1.0)
            nxt = (tg, hhg + 1) if hhg < 3 else (tg + 1, 0)
            if nxt[0] < 2 and nxt not in pre["htt"]:
                pre["htt"][nxt] = htt_tile(*nxt)
            for hhi in range(8):
                hh = hhg * 8 + hhi
                w2 = pre["w2"].pop(0) if (tg == 0 and hh < 3) else w2_tile(hh)
                for tt in range(4):
                    pr, ph = tt // 2, tt % 2
                    for half in range(2):
                        nc.tensor.matmul(
                            pss[tt][:, half * 512:(half + 1) * 512],
                            htg[:, pr, hhi, ph * 128:(ph + 1) * 128],
                            w2[:, half * 512:(half + 1) * 512],
                            start=(hh == 0), stop=(hh == 31),
                        )
        for tt in range(4):
            pos = tg * 4 + tt
            yt = yp.tile([128, D], F32, tag="yt")
            nc.vector.tensor_add(out=yt[:], in0=pss[tt][:], in1=x2in[tt][:])
            nc.sync.dma_start(
                y_d.ap()[pos * 128:(pos + 1) * 128, :], yt[:])

    ff2ps.release()
    yp.release()
    x2ip.release()
    htgp.release()
    htp.release()
    w2p.release()
    w1_holder["pb2"].release()
    w1_holder["pb1"].release()
    w1_holder["pa"].release()
    hst_p.release()
    x2t_pool.release()
    p7.release()
    epp.release()
    atp.release()
    attt_pool.release()
    att_pool.release()
    xt2_p.release()
    wqk_p.release()
    pw_p.release()


# ---------------------------------------------------------------------------
# Host wrapper
# ---------------------------------------------------------------------------

_PROG_CACHE = {}


def _get_program(niter=None):
    if niter not in _PROG_CACHE:
        _PROG_CACHE[niter] = build_program(niter)
    return _PROG_CACHE[niter]


def make_in_maps(x, ln1_g, ln1_b, qkv_w, qkv_b, proj_w, proj_b,
                 ln2_g, ln2_b, ff1_w, ff1_b, ff2_w, ff2_b):
    bf = ml_dtypes.bfloat16
    f32 = np.float32

    def pcol(v, n):
        return np.ascontiguousarray(np.asarray(v, f32).reshape(n, 128).T)

    def dimmajor(w, nchunk, ncol):
        return np.ascontiguousarray(
            np.asarray(w, f32).reshape(nchunk, 128, ncol).transpose(1, 0, 2)
        ).astype(bf)

    # fold LN1 gain/bias into the QKV weights and LN2 gain/bias into FF1:
    # LN(x)*g + b feeding W  ==  LN_raw(x) @ (g[:,None]*W) + (b @ W + bias).
    # Device-side LN then omits gain/bias and the transpose copies are pure.
    qkv_w0 = np.asarray(qkv_w, f32)
    g1v, b1v = np.asarray(ln1_g, f32), np.asarray(ln1_b, f32)
    qkv_b = np.asarray(qkv_b, f32) + b1v @ qkv_w0
    qkv_w = g1v[:, None] * qkv_w0
    ff1_w0 = np.asarray(ff1_w, f32)
    g2v, b2v = np.asarray(ln2_g, f32), np.asarray(ln2_b, f32)
    ff1_b = np.asarray(ff1_b, f32) + b2v @ ff1_w0
    ff1_w = g2v[:, None] * ff1_w0
    common = dict(
        wqkv=dimmajor(qkv_w, 8, 3 * D),
        wproj=dimmajor(proj_w, 8, D),
        wff1=dimmajor(ff1_w, 8, HID),
        wff2=dimmajor(ff2_w, 32, D),
        bqk=np.ascontiguousarray(
            np.concatenate([pcol(qkv_b[0:D], 8), pcol(qkv_b[D:2 * D], 8)], axis=1)),
        bv=qkv_b[2 * D:3 * D].copy(),
        bff1=pcol(ff1_b, 32),
        bff2b=np.asarray(ff2_b, f32).astype(bf),
    )
    masks = [np.ascontiguousarray(_masks(p).transpose(1, 0, 2)).astype(bf)
             for p in range(2)]
    perms = [_perm(0), _perm(1)]

    projb = np.asarray(proj_b, f32)
    x = np.asarray(x, f32)
    in_maps = []
    for c in range(N_CORES):
        b, p = c // 2, c % 2
        m = dict(common)
        m["x_full"] = np.ascontiguousarray(x[b]).astype(bf)
        m["x_q"] = np.ascontiguousarray(x[b][perms[p]]).astype(bf)
        m["x_qr"] = (m["x_q"] + projb[None, :]).astype(bf)
        m["mask"] = masks[p]
        in_maps.append(m)
    return in_maps, perms


def kernel(**inputs):
    in_maps, perms = make_in_maps(**{k: np.asarray(v) for k, v in inputs.items()})
    nc = _get_program()
    res = run_bass_kernel_spmd(nc, in_maps, list(range(N_CORES))).results
    y = np.empty((B, T, D), np.float32)
    for c in range(N_CORES):
        b, p = c // 2, c % 2
        y[b][perms[p]] = res[c]["y"]
    return y



# revision 83
# speedup vs baseline: 1.0137x; 1.0137x over previous
"""Trainium2 Bass kernel v3 for a dense transformer block (B=4, T=2048, D=1024, H=16).

Sharding: 8 cores = 4 batches x 2 query-parity groups (zigzag), slots
processed in mixed order ORDER = [7,0,1,6,2,3,5,4] (NK blocks
[2,16,14,4,12,10,6,8]) so that:
  - K/V block-pair pools release progressively (big slots early) while FF1
    state grows -- complementary SBUF footprints;
  - exp (ACT-bound) of big slots overlaps FF1/proj/tails (PE-dense fillers).

Structure per core:
  prefix:   x DMA -> LN1 -> transpose (pipelined per 4-block group; the first
            x tiles lead the weight streams), V/K/Q production interleaved.
  attn:     per position: scores (head-paired K=64, row-tiled) -> exp(ACT) ->
            mask(Pool) -> PV (ones-column denominators) -> scale -> att;
            tail: transpose, proj, + bf16 residual (proj bias folded host-
            side) -> x2, LN2 -> XT2. xq residual tiles prefetched one tail
            ahead. FF1 interleaved as PE filler; gelu -> stage -> HT to DRAM.
  ff2:      streams HT (prefetched one hhg ahead) + W2 (10-deep ring) from
            DRAM, 8-bank PSUM accumulate; ff2 bias folded into x2 reload on
            Pool so the tail is a single DVE add per tile.
Queue isolation (x/xq/weights on ACT, masks on Pool, HT/W2/y on SP) lets the
next iteration's LN1 prefix overlap this iteration's FF2 phase.
"""

import sys

try:
    import concourse  # noqa: F401
except ImportError:
    sys.path.insert(0, "/opt/trn_rl_repo")

import numpy as np
import ml_dtypes

import concourse.bass as bass
import concourse.bacc as bacc
import concourse.tile as tile
from concourse.tile import add_dep_helper
from concourse import mybir
from concourse.bass_utils import run_bass_kernel_spmd

F32 = mybir.dt.float32
BF16 = mybir.dt.bfloat16
AF = mybir.ActivationFunctionType
ALU = mybir.AluOpType
PSUM = bass.MemorySpace.PSUM

B, T, D = 4, 2048, 1024
H, HD = 16, 64
HID = 4 * D
EPS = 1e-5
N_CORES = 8
NB = T // 128  # 16 token blocks per batch
NQ = 8  # query slots per core
TQ = NQ * 128  # query tokens per core

# zigzag query-block assignment (parity balances causal work)
QB = [
    [15, 12, 11, 8, 7, 4, 3, 0],
    [14, 13, 10, 9, 6, 5, 2, 1],
]
NKMAX = [16, 14, 12, 10, 8, 6, 4, 2]
# processing order: position -> zigzag slot index. 2-blk slot first (its Q/K
# deferral fillers run under it), then big slots early so kv pools release
# while FF1 state grows.
ORDER = [7, 0, 1, 6, 2, 3, 5, 4]
NK = [NKMAX[o] for o in ORDER]  # [2, 16, 14, 4, 12, 10, 6, 8]


def _perm(p):
    idx = []
    for pos in range(NQ):
        qb = QB[p][ORDER[pos]]
        idx.extend(range(qb * 128, (qb + 1) * 128))
    return np.array(idx, dtype=np.int64)


def _masks(p):
    """[NQ, 128, 256] multiplicative mask for the last two key blocks per pos."""
    m = np.zeros((NQ, 128, 256), dtype=np.float32)
    tri = np.triu(np.ones((128, 128), np.float32))  # valid where k_local <= q_local
    for pos in range(NQ):
        nk = QB[p][ORDER[pos]] + 1
        nkm = NK[pos]
        for c in range(2):
            ki = nkm - 2 + c
            if ki < nk - 1:
                m[pos, :, c * 128:(c + 1) * 128] = 1.0
            elif ki == nk - 1:
                m[pos, :, c * 128:(c + 1) * 128] = tri
    return m


# ---------------------------------------------------------------------------
# Device program
# ---------------------------------------------------------------------------

def _layernorm_tile(nc, pool, x_ap, out_ap, eps_t, norm_engine=None):
    """LN (without gain/bias) of a [128, D] tile -> out_ap bf16."""
    stats = pool.tile([128, 2, nc.vector.BN_STATS_DIM], F32, tag="ln_stats")
    mv = pool.tile([128, nc.vector.BN_AGGR_DIM], F32, tag="ln_mv")
    nmr = pool.tile([128, 1], F32, tag="ln_nmr")
    xg = x_ap.rearrange("p (s d) -> p s d", s=2)
    for s in range(2):
        nc.vector.bn_stats(out=stats[:, s, :], in_=xg[:, s, :])
    nc.vector.bn_aggr(out=mv[:], in_=stats[:])
    mean = mv[:, 0:1]
    rstd = mv[:, 1:2]
    nc.scalar.activation(out=rstd, in_=rstd, func=AF.Sqrt, bias=eps_t[:], scale=1.0)
    nc.vector.reciprocal(out=rstd, in_=rstd)
    nc.vector.tensor_scalar(
        out=nmr[:], in0=mean, scalar1=rstd, scalar2=-1.0,
        op0=ALU.mult, op1=ALU.mult,
    )
    if norm_engine == "vector":
        nc.vector.tensor_scalar(
            out=out_ap, in0=x_ap, scalar1=rstd, scalar2=nmr[:],
            op0=ALU.mult, op1=ALU.add,
        )
    else:
        nc.scalar.activation(out=out_ap, in_=x_ap, func=AF.Identity,
                             bias=nmr[:], scale=rstd)


def build_program(niter=None, stop_after=None):
    nc = bacc.Bacc("TRN2", target_bir_lowering=False, debug=False,
                   num_devices=N_CORES)

    xf_d = nc.dram_tensor("x_full", [T, D], BF16, kind="ExternalInput")
    xq_d = nc.dram_tensor("x_q", [TQ, D], BF16, kind="ExternalInput")
    xqr_d = nc.dram_tensor("x_qr", [TQ, D], BF16, kind="ExternalInput")
    mask_d = nc.dram_tensor("mask", [128, NQ, 256], BF16, kind="ExternalInput")
    wqkv_d = nc.dram_tensor("wqkv", [128, 8, 3 * D], BF16, kind="ExternalInput")
    wproj_d = nc.dram_tensor("wproj", [128, 8, D], BF16, kind="ExternalInput")
    wff1_d = nc.dram_tensor("wff1", [128, 8, HID], BF16, kind="ExternalInput")
    wff2_d = nc.dram_tensor("wff2", [128, 32, D], BF16, kind="ExternalInput")
    bqk_d = nc.dram_tensor("bqk", [128, 16], F32, kind="ExternalInput")
    bv_d = nc.dram_tensor("bv", [D], F32, kind="ExternalInput")
    bff1_d = nc.dram_tensor("bff1", [128, 32], F32, kind="ExternalInput")
    bff2b_d = nc.dram_tensor("bff2b", [D], BF16, kind="ExternalInput")
    y_d = nc.dram_tensor("y", [TQ, D], F32, kind="ExternalOutput")
    ht_d = nc.dram_tensor("ht_scratch", [128, 4, 32, 256], BF16, kind="Internal")
    x2_d = nc.dram_tensor("x2_scratch", [TQ, D], BF16, kind="Internal")

    def bcast_row(dram_handle, n):
        ap = dram_handle.ap()
        return bass.AP(tensor=ap.tensor, offset=ap.offset, ap=[[0, 128], [1, n]])

    dram = dict(xf=xf_d, xq=xq_d, xqr=xqr_d, wqkv=wqkv_d, wproj=wproj_d,
                wff1=wff1_d, wff2=wff2_d, y=y_d, ht=ht_d, x2=x2_d)

    with tile.TileContext(nc) as tc:
        with tc.tile_pool(name="singles", bufs=1) as singles:
            ident = singles.tile([128, 128], BF16)
            from concourse.masks import make_identity
            make_identity(nc, ident[:])
            nc._ident = ident

            st = {}
            st["eps"] = singles.tile([128, 1], F32, name="eps", tag="eps")
            nc.vector.memset(st["eps"][:], EPS)
            for nm, dt, shape, src in (
                ("bqk", F32, [128, 16], bqk_d), ("bff1", F32, [128, 32], bff1_d),
                ("mask", BF16, [128, NQ, 256], mask_d),
            ):
                st[nm] = singles.tile(shape, dt, name=nm, tag=nm)
                nc.sync.dma_start(st[nm][:], src.ap())
            st["fb2"] = singles.tile([128, D], BF16, name="fb2", tag="fb2")
            nc.gpsimd.dma_start(st["fb2"][:], bcast_row(bff2b_d, D))
            nc._bcast_row = bcast_row
            nc._bv_d = bv_d

            if niter is None:
                _block_body(tc, nc, dram, st, stop_after)
            elif isinstance(niter, str) and niter.startswith("unroll"):
                for _ in range(int(niter[6:])):
                    _block_body(tc, nc, dram, st, stop_after)
            else:
                with tc.For_i(0, niter, 1):
                    _block_body(tc, nc, dram, st, stop_after)

    nc.compile()
    return nc


def _block_body(tc, nc, dram, st, stop_after=None):
    xf_d, xq_d, y_d = dram["xf"], dram["xq"], dram["y"]
    ht_d, x2_d = dram["ht"], dram["x2"]
    eps_t = st["eps"]

    # ---- persistent pools --------------------------------------------------
    # right side (bottom->top): qt, kv[0..7], xtq, xtf
    qt_p = tc.alloc_tile_pool(name="qt", bufs=1, side="right")
    QT = qt_p.tile([128, 8, TQ], BF16, name="QT", tag="qt")
    kv = []
    kvt = []
    for b in range(8):
        p = tc.alloc_tile_pool(name=f"kv{b}", bufs=1, side="right")
        KT2 = p.tile([128, 8, 256], BF16, name=f"KT{b}", tag="kt")
        V2 = p.tile([128, 2, H, HD + 1], BF16, name=f"V{b}", tag="v")
        nc.vector.memset(V2[:, :, :, HD:HD + 1], 1.0)
        kv.append(p)
        kvt.append((KT2, V2))
    xtq_p = tc.alloc_tile_pool(name="xtq", bufs=1, side="right")
    XTQ = xtq_p.tile([128, 8, TQ], BF16, name="XTQ", tag="xtq")
    xtf_p = tc.alloc_tile_pool(name="xtf", bufs=1, side="right")
    XTF = xtf_p.tile([128, 8, T], BF16, name="XTF", tag="xtf")

    def KTb(kk, po, hc):  # [64, 128] stationary view of key block kk
        return kvt[kk // 2][0][po:po + 64, hc, (kk % 2) * 128:(kk % 2 + 1) * 128]

    def Vb(kk, h):  # [128, HD+1] view of key block kk, head h
        return kvt[kk // 2][1][:, kk % 2, h, :]

    pw_p = tc.alloc_tile_pool(name="wproj", bufs=1, side="left")
    PW = pw_p.tile([128, 8, D], BF16, name="PW", tag="pw")

    # ---- prefix: fused LN1 + transpose + V/K/Q production ------------------
    wqk_p = tc.alloc_tile_pool(name="wqk", bufs=3, side="left")
    p1 = tc.alloc_tile_pool(name="p1", bufs=6, side="left")
    p1s = tc.alloc_tile_pool(name="p1s", bufs=8, side="left")
    xln_p = tc.alloc_tile_pool(name="xln", bufs=6, side="left")
    wv_p = tc.alloc_tile_pool(name="wv", bufs=1, side="left")
    pre_tp = tc.alloc_tile_pool(name="pre_tp", bufs=2, space=PSUM)
    pre_mm = tc.alloc_tile_pool(name="pre_mm", bufs=2, space=PSUM)

    def ln_group(x_d, t0, n, XT):
        """LN + transpose n token blocks starting at t0 from x_d into XT."""
        tiles = []
        for u in range(n):
            t = t0 + u
            xt = p1.tile([128, D], BF16, tag="xt", name="xt")
            q = nc.gpsimd if (x_d is xf_d and u % 2 == 0) else nc.scalar
            q.dma_start(xt[:], x_d.ap()[t * 128:(t + 1) * 128, :])
            ln = xln_p.tile([128, D], BF16, tag="ln", name="ln")
            _layernorm_tile(nc, p1s, xt[:], ln[:], eps_t)
            tiles.append(ln)
        for dc in range(8):
            pt = pre_tp.tile([128, n * 128], BF16, tag="tp", name="pt",
                             padded_shape=[128, 1024])
            for u in range(n):
                nc.tensor.transpose(
                    pt[:, u * 128:(u + 1) * 128],
                    tiles[u][:, dc * 128:(dc + 1) * 128], nc._ident[:])
            dst = XT[:, dc, t0 * 128:(t0 + n) * 128]
            if dc % 2 == 1:
                nc.scalar.copy(out=dst, in_=pt[:])
            else:
                nc.vector.tensor_copy(dst, pt[:])

    def emit_v(tg, wv_tiles, vb_t):
        """V for token blocks 4tg..4tg+3 (needs XTF of those blocks)."""
        for vh in range(2):
            for u in range(4):
                t = tg * 4 + u
                ps = pre_mm.tile([128, 512], F32, tag="mm", name="ps")
                for dc in range(8):
                    nc.tensor.matmul(
                        ps[:], XTF[:, dc, t * 128:(t + 1) * 128],
                        wv_tiles[vh][:, dc, :], start=(dc == 0), stop=(dc == 7),
                    )
                nc.vector.tensor_add(
                    out=kvt[t // 2][1][:, t % 2, vh * 8:(vh + 1) * 8, 0:HD],
                    in0=ps[:].rearrange("p (h d) -> p h d", h=8),
                    in1=vb_t[:, vh * 512:(vh + 1) * 512]
                        .rearrange("p (h d) -> p h d", h=8),
                )

    def emit_k(cc, pairs):
        """KT chunk occ=cc-8 for token pair(s) — one weight fetch shared
        across all pairs whose XTF is ready."""
        occ = cc - 8
        w = wqk_p.tile([128, 8, 128], BF16, tag="wqk", name="w")
        q = nc.sync if cc % 2 == 0 else nc.gpsimd
        q.dma_start(
            w[:], dram["wqkv"].ap()[:, :, cc * 128:(cc + 1) * 128])
        for pair in pairs:
            pshs = [pre_mm.tile([128, 512], F32, tag="mm", name="ps")
                    for _ in range(2)]
            for h2 in range(2):
                for dc in range(8):
                    nc.tensor.matmul(
                        pshs[h2][:],
                        w[:, dc, :],
                        XTF[:, dc, (pair * 2 + h2) * 512:
                            (pair * 2 + h2 + 1) * 512],
                        start=(dc == 0), stop=(dc == 7),
                    )
            for g in range(4):
                b = pair * 4 + g
                nc.vector.tensor_scalar_add(
                    out=kvt[b][0][:, occ, :],
                    in0=pshs[g // 2][:, (g % 2) * 256:(g % 2 + 1) * 256],
                    scalar1=st["bqk"][:, cc:cc + 1],
                )

    def emit_q_prefix(cc):
        """Q chunk cc for positions 0-1 (cols 0-255) only."""
        w = wqk_p.tile([128, 8, 128], BF16, tag="wqk", name="w")
        q = nc.sync if cc % 2 == 0 else nc.gpsimd
        q.dma_start(
            w[:], dram["wqkv"].ap()[:, :, cc * 128:(cc + 1) * 128])
        ps = pre_mm.tile([128, 256], F32, tag="mm", name="ps",
                         padded_shape=[128, 512])
        for dc in range(8):
            nc.tensor.matmul(
                ps[:], w[:, dc, :], XTQ[:, dc, 0:256],
                start=(dc == 0), stop=(dc == 7),
            )
        nc.vector.tensor_scalar_add(
            out=QT[:, cc, 0:256], in0=ps[:], scalar1=st["bqk"][:, cc:cc + 1],
        )

    # pipelined prefix: x DMAs lead (weight streams issued after the first
    # LN groups so the first tiles aren't queued behind bulk weights)
    wv_tiles = [wv_p.tile([128, 8, 512], BF16, tag=f"wv{vh}", name=f"wv{vh}")
                for vh in range(2)]
    vb_t = wv_p.tile([128, D], F32, name="vb", tag="vb")

    ln_group(xf_d, 0, 2, XTF)
    for vh in range(2):
        nc.sync.dma_start(
            wv_tiles[vh][:], dram["wqkv"].ap()[:, :, 2 * D + vh * 512:
                                               2 * D + (vh + 1) * 512])
    nc.gpsimd.dma_start(vb_t[:], nc._bcast_row(nc._bv_d, D))
    ln_group(xf_d, 2, 2, XTF)
    ln_group(xf_d, 4, 4, XTF)
    emit_v(0, wv_tiles, vb_t)
    ln_group(xf_d, 8, 4, XTF)
    emit_v(1, wv_tiles, vb_t)
    for cc in range(8, 12):
        emit_k(cc, (0,))
    ln_group(xf_d, 12, 4, XTF)
    for cc in range(12, 16):
        emit_k(cc, (0, 1))
    emit_v(2, wv_tiles, vb_t)
    ln_group(xq_d, 0, 4, XTQ)
    emit_v(3, wv_tiles, vb_t)
    for cc in range(8, 12):
        emit_k(cc, (1,))
    ln_group(xq_d, 4, 4, XTQ)
    for cc in range(8):
        emit_q_prefix(cc)

    pre_mm.release()
    pre_tp.release()
    wv_p.release()
    xln_p.release()
    p1s.release()
    p1.release()
    xtf_p.release()
    if stop_after == "prefix":
        xtq_p.release()
        wqk_p.release()
        for b in reversed(range(8)):
            kv[b].release()
        qt_p.release()
        pw_p.release()
        return

    # ---- attention + interleaved FF1 --------------------------------------
    xt2_p = tc.alloc_tile_pool(name="xt2", bufs=1, side="left")
    XT2P = [xt2_p.tile([128, 8, 256], BF16, name=f"XT2_{q}", tag=f"xt2{q}")
            for q in range(4)]

    sc_ps = tc.alloc_tile_pool(name="sc_ps", bufs=2, space=PSUM)
    pv_ps = tc.alloc_tile_pool(name="pv_ps", bufs=1, space=PSUM)
    tp_ps = tc.alloc_tile_pool(name="tp_ps", bufs=1, space=PSUM)
    pj_ps = tc.alloc_tile_pool(name="pj_ps", bufs=1, space=PSUM)
    f1_ps = tc.alloc_tile_pool(name="f1_ps", bufs=1, space=PSUM)

    att_pool = tc.alloc_tile_pool(name="attsl", bufs=2, side="left")
    attt_pool = tc.alloc_tile_pool(name="atttsl", bufs=1, side="left")
    atp = tc.alloc_tile_pool(name="at", bufs=5, side="left")
    epp = tc.alloc_tile_pool(name="ep", bufs=2, side="left")
    p7 = tc.alloc_tile_pool(name="p7", bufs=1, side="left")
    x2t_pool = tc.alloc_tile_pool(name="x2t", bufs=2, side="left")
    hst_p = tc.alloc_tile_pool(name="hstage", bufs=1, side="left")
    w1_holder = {}

    def emit_scores_pair(hc, pos, att_j):
        """Scores+exp+mask for BOTH heads of pair hc at position pos."""
        nblk = NK[pos]
        out = []
        for hp in range(2):
            out.append((2 * hc + hp, pos, [], att_j))
        for g in range((nblk + 7) // 8):
            blo = g * 8
            bhi = min(blo + 8, nblk)
            ncol = (bhi - blo) * 128
            pss = [sc_ps.tile([128, 1024], F32, tag="sc", name="ps")
                   for _ in range(2)]
            for kk in range(blo, bhi):
                for hp in range(2):
                    po = hp * 64
                    nc.tensor.matmul(
                        pss[hp][:, (kk - blo) * 128:(kk - blo + 1) * 128],
                        KTb(kk, po, hc),
                        QT[po:po + 64, hc, pos * 128:(pos + 1) * 128],
                        start=True, stop=True,
                    )
            for hp in range(2):
                at = atp.tile([128, 1024], BF16, tag="at", name="at")
                nc.scalar.activation(out=at[:, 0:ncol], in_=pss[hp][:, 0:ncol],
                                     func=AF.Exp, scale=0.125)
                for kk in range(max(blo, nblk - 2), bhi):
                    mc = (kk - (nblk - 2)) * 128
                    nc.gpsimd.tensor_mul(
                        out=at[:, (kk - blo) * 128:(kk - blo + 1) * 128],
                        in0=at[:, (kk - blo) * 128:(kk - blo + 1) * 128],
                        in1=st["mask"][:, pos, mc:mc + 128],
                    )
                out[hp][2].append((blo, bhi, at))
        return out

    def emit_pv_pair(pend2):
        """PV for both heads of a pair into one [128, 2, HD+1] PSUM tile."""
        pv = pv_ps.tile([128, 2, HD + 1], F32, tag="pv", name="pv")
        for hp, pend in enumerate(pend2):
            h, pos, ats, att_j = pend
            nblk = NK[pos]
            for blo, bhi, at in ats:
                for kk in range(blo, bhi):
                    nc.tensor.matmul(
                        pv[:, hp, :],
                        at[:, (kk - blo) * 128:(kk - blo + 1) * 128],
                        Vb(kk, h),
                        start=(kk == 0), stop=(kk == nblk - 1),
                    )
        for hp, pend in enumerate(pend2):
            h, pos, ats, att_j = pend
            r = epp.tile([128, 1], F32, tag="recip", name="r")
            nc.vector.reciprocal(out=r[:], in_=pv[:, hp, HD:HD + 1])
            nc.vector.tensor_scalar_mul(
                out=att_j[:, h * HD:(h + 1) * HD],
                in0=pv[:, hp, 0:HD], scalar1=r[:],
            )

    xq_pre = {}

    def fetch_xq(pos):
        t = p7.tile([128, D], BF16, tag="xq", name="xq", bufs=2)
        nc.sync.dma_start(t[:], dram["xqr"].ap()[pos * 128:(pos + 1) * 128, :])
        return t

    def slot_tail(pos, att_j):
        """att -> transpose -> proj -> +bias +resid -> x2t; LN2 -> XT2; spill."""
        attt = attt_pool.tile([128, 8, 128], BF16, tag="attt", name="attt")
        for g2 in range(2):
            pt = tp_ps.tile([128, 512], BF16, tag="tp", name="pt",
                            padded_shape=[128, 1024])
            for u in range(4):
                dc = g2 * 4 + u
                nc.tensor.transpose(
                    pt[:, u * 128:(u + 1) * 128],
                    att_j[:, dc * 128:(dc + 1) * 128], nc._ident[:])
            nc.vector.tensor_copy(attt[:, g2 * 4:(g2 + 1) * 4, :], pt[:])
        xq = xq_pre.pop(pos, None)
        if xq is None:
            xq = fetch_xq(pos)
        if pos + 1 < NQ and pos + 1 not in xq_pre:
            xq_pre[pos + 1] = fetch_xq(pos + 1)
        x2t = x2t_pool.tile([128, D], BF16, tag="x2t", name="x2t")
        for half in range(2):
            ps = pj_ps.tile([128, 512], F32, tag="pj", name="ps")
            for hcc in range(8):
                nc.tensor.matmul(
                    ps[:],
                    attt[:, hcc, :],
                    PW[:, hcc, half * 512:(half + 1) * 512],
                    start=(hcc == 0), stop=(hcc == 7),
                )
            nc.vector.tensor_add(
                out=x2t[:, half * 512:(half + 1) * 512],
                in0=ps[:], in1=xq[:, half * 512:(half + 1) * 512])
        x2_out_insts[pos] = nc.sync.dma_start(
            x2_d.ap()[pos * 128:(pos + 1) * 128, :], x2t[:])
        # LN2 on x2t -> bf16, then transpose into the position's XT2 pair tile
        ln2 = p7.tile([128, D], BF16, tag="ln2", name="ln2")
        _layernorm_tile(nc, epp, x2t[:], ln2[:], eps_t, norm_engine="vector")
        pr, qcol = pos // 2, (pos % 2) * 128
        for g2 in range(2):
            pt = tp_ps.tile([128, 512], BF16, tag="tp", name="pt",
                            padded_shape=[128, 1024])
            for u in range(4):
                dc = g2 * 4 + u
                nc.tensor.transpose(
                    pt[:, u * 128:(u + 1) * 128],
                    ln2[:, dc * 128:(dc + 1) * 128], nc._ident[:])
            nc.vector.tensor_copy(
                XT2P[pr][:, g2 * 4:(g2 + 1) * 4, qcol:qcol + 128],
                pt[:].rearrange("p (a b) -> p a b", a=4))

    def w_fetch(cc):
        w = wqk_p.tile([128, 8, 128], BF16, tag="wqk", name="w")
        nc.sync.dma_start(
            w[:], dram["wqkv"].ap()[:, :, cc * 128:(cc + 1) * 128])
        return w

    def q_filler(cc, w):
        """Deferred Q production: chunk cc, column blocks 1-3 (768 cols)."""
        for cb in range(1, 4):
            ps = f1_ps.tile([128, 2, 256], F32, tag="f1", name="ps")
            for dc in range(8):
                nc.tensor.matmul(
                    ps[:, 0, :], w[:, dc, :],
                    XTQ[:, dc, cb * 256:(cb + 1) * 256],
                    start=(dc == 0), stop=(dc == 7),
                )
            nc.vector.tensor_scalar_add(
                out=QT[:, cc, cb * 256:(cb + 1) * 256], in0=ps[:, 0, :],
                scalar1=st["bqk"][:, cc:cc + 1],
            )

    def ff1_chunk(pair, hhg):
        """FF1 hidden group hhg (8 hh) for position pair (256 tokens)."""
        if hhg < 2:
            W1, hoff = w1_holder["a"], 0
        elif hhg == 2:
            W1, hoff = w1_holder["b1"], 16
        else:
            W1, hoff = w1_holder["b2"], 24
        stage = hst_p.tile([128, 8, 256], BF16, tag="hs", name="hs")
        for hhi in range(0, 8, 2):
            if pair == 3:
                # post-attention: pv/tp/pj banks are dead — cycle all four
                # 1-bank pools so the MMs never stall on the stage copies
                pool, tag = ((f1_ps, "f1"), (pv_ps, "pv"),
                             (tp_ps, "tp"), (pj_ps, "pj"))[hhi // 2]
                ps = pool.tile([128, 2, 256], F32, tag=tag, name="ps")
            else:
                ps = f1_ps.tile([128, 2, 256], F32, tag="f1", name="ps")
            for c in range(2):
                hh = hhg * 8 + hhi + c
                for dc in range(8):
                    nc.tensor.matmul(
                        ps[:, c, :],
                        W1[:, dc, (hh - hoff) * 128:(hh - hoff + 1) * 128],
                        XT2P[pair][:, dc, :],
                        start=(dc == 0), stop=(dc == 7),
                    )
            if hhi % 4 == 0 or pair == 3:
                nc.vector.tensor_copy(stage[:, hhi:hhi + 2, :], ps[:])
            else:
                nc.scalar.copy(out=stage[:, hhi:hhi + 2, :], in_=ps[:])
        ht_out_insts[(pair, hhg)] = nc.sync.dma_start(
            ht_d.ap()[:, pair, hhg * 8:(hhg + 1) * 8, :], stage[:])

    ht_out_insts = {}
    x2_out_insts = {}

    # filler queue: (weight_cc_or_None, compute) pairs, ~3-7us of PE work
    # each; weight DMAs prefetched 3 fillers ahead via the wqk ring.
    fillers = []
    for i in range(8):
        fillers.append((i, lambda w, c=i: q_filler(c, w)))
    prefetched = []

    def _prime():
        while len(prefetched) < 3 and fillers:
            cc2, fn2 = fillers.pop(0)
            prefetched.append((fn2, w_fetch(cc2) if cc2 is not None else None))

    _prime()

    def drain_filler(k=1):
        for _ in range(k):
            _prime()
            if not prefetched:
                return
            fn, w = prefetched.pop(0)
            fn(w)

    pending = None
    done = []
    wload_sched = {}
    for qtr in range(4):
        wload_sched[(0, qtr)] = (
            lambda q=qtr: nc.sync.dma_start(
                PW[:, 2 * q:2 * q + 2, :],
                dram["wproj"].ap()[:, 2 * q:2 * q + 2, :]))
    for pos in range(NQ):
        att_j = att_pool.tile([128, D], BF16, tag="att", name="att_j")
        for hc in range(8):
            if (pos, hc) in wload_sched:
                wload_sched.pop((pos, hc))()
            cur2 = emit_scores_pair(hc, pos, att_j)
            if pending is not None:
                emit_pv_pair(pending)
            pending = cur2
            if hc == 0 and done:
                slot_tail(*done.pop())
            elif pos < 2:
                drain_filler(2)
            elif pos < 4:
                if hc in (2, 4, 6):
                    drain_filler(2 if len(fillers) + len(prefetched) > 1 else 1)
            else:
                drain_filler(1)
        done.append((pos, att_j))
        if pos == 0:
            assert not fillers and not prefetched  # deferred Q done
            xtq_p.release()
            # W1 piece A (hh 0-15): allocate now (reusing XTQ's bytes) but
            # stream the halves mid-pos1 on sync, clear of the pos0 tail's
            # xq/x2 DMAs and of Pool's mask path.
            w1a_p = tc.alloc_tile_pool(name="w1a", bufs=1, side="left")
            W1A = w1a_p.tile([128, 8, HID // 2], BF16, name="W1A", tag="w1a")
            wload_sched[(1, 2)] = lambda: nc.sync.dma_start(
                W1A[:, :, 0:HID // 4], dram["wff1"].ap()[:, :, 0:HID // 4])
            wload_sched[(1, 5)] = lambda: nc.sync.dma_start(
                W1A[:, :, HID // 4:HID // 2],
                dram["wff1"].ap()[:, :, HID // 4:HID // 2])
            w1_holder["a"] = W1A
            w1_holder["pa"] = w1a_p
        if pos == 1:
            kv[7].release()
            # W1 piece B1 (hh 16-23), streamed mid-pos2
            w1b1_p = tc.alloc_tile_pool(name="w1b1", bufs=1, side="left")
            W1B1 = w1b1_p.tile([128, 8, HID // 4], BF16, name="W1B1",
                               tag="w1b1")
            wload_sched[(2, 3)] = lambda: nc.sync.dma_start(
                W1B1[:], dram["wff1"].ap()[:, :, HID // 2:3 * HID // 4])
            w1_holder["b1"] = W1B1
            w1_holder["pb1"] = w1b1_p
        if pos == 2:
            kv[6].release()
            # W1 piece B2 (hh 24-31), streamed mid-pos3
            w1b2_p = tc.alloc_tile_pool(name="w1b2", bufs=1, side="left")
            W1B2 = w1b2_p.tile([128, 8, HID // 4], BF16, name="W1B2",
                               tag="w1b2")
            wload_sched[(3, 3)] = lambda: nc.sync.dma_start(
                W1B2[:], dram["wff1"].ap()[:, :, 3 * HID // 4:])
            w1_holder["b2"] = W1B2
            w1_holder["pb2"] = w1b2_p
            for hhg in (0, 1):
                fillers.append((None, lambda w, h_=hhg: ff1_chunk(0, h_)))
        if pos == 3:
            for hhg in (2, 3):
                fillers.append((None, lambda w, h_=hhg: ff1_chunk(0, h_)))
        if pos == 4:
            for hhg in range(4):
                fillers.append((None, lambda w, h_=hhg: ff1_chunk(1, h_)))
        if pos == 6:
            for hhg in range(4):
                fillers.append((None, lambda w, h_=hhg: ff1_chunk(2, h_)))
    emit_pv_pair(pending)
    slot_tail(*done.pop())
    while fillers or prefetched:
        drain_filler()

    for b in (5, 4, 3, 2, 1, 0):
        kv[b].release()
    qt_p.release()

    # FF2 pools + tg0 prefetch issued BEFORE the last FF1 pair so the sync
    # queue has tg0's streams in flight while pair-3 FF1 finishes on PE.
    w2p = tc.alloc_tile_pool(name="w2", bufs=10, side="left")
    htp = tc.alloc_tile_pool(name="htin", bufs=2, side="left")
    htgp = tc.alloc_tile_pool(name="htgel", bufs=2, side="left")
    x2ip = tc.alloc_tile_pool(name="x2in", bufs=4, side="left")
    yp = tc.alloc_tile_pool(name="yp", bufs=2, side="left")

    pre = {}

    def x2_tile(pos):
        x2i = x2ip.tile([128, D], BF16, tag="x2i", name="x2i")
        rd = nc.sync.dma_start(x2i[:], x2_d.ap()[pos * 128:(pos + 1) * 128, :])
        add_dep_helper(rd.ins, x2_out_insts[pos].ins, True,
                       "x2 scratch RAW across DMA queues")
        # fold the ff2 bias in on Pool (idle during FF2) so the tail is one add
        nc.gpsimd.tensor_add(out=x2i[:], in0=x2i[:], in1=st["fb2"][:])
        return x2i

    def htt_tile(tg, hhg):
        htt = htp.tile([128, 2, 8, 256], BF16, tag="ht", name="htt")
        rd = nc.sync.dma_start(
            htt[:], ht_d.ap()[:, 2 * tg:2 * tg + 2, hhg * 8:(hhg + 1) * 8, :])
        for pr in (2 * tg, 2 * tg + 1):
            add_dep_helper(rd.ins, ht_out_insts[(pr, hhg)].ins, True,
                           "ht scratch RAW across DMA queues")
        return htt

    def w2_tile(hh):
        w2 = w2p.tile([128, D], BF16, tag="w2")
        nc.sync.dma_start(w2[:], dram["wff2"].ap()[:, hh, :])
        return w2

    pre["x2"] = [x2_tile(pos) for pos in range(4)]
    pre["htt"] = {(0, 0): htt_tile(0, 0)}
    pre["w2"] = [w2_tile(hh) for hh in range(3)]

    for hhg in range(4):
        ff1_chunk(3, hhg)

    f1_ps.release()
    pj_ps.release()
    tp_ps.release()
    pv_ps.release()
    sc_ps.release()
    ff2ps = tc.alloc_tile_pool(name="ff2ps", bufs=4, space=PSUM)

    for tg in range(2):
        x2in = pre.pop("x2") if tg == 0 else [x2_tile(tg * 4 + tt)
                                              for tt in range(4)]
        pss = [ff2ps.tile([128, 1024], F32, name="ym", tag="ym")
               for _ in range(4)]
        for hhg in range(4):
            htt = pre["htt"].pop((tg, hhg), None)
            if htt is None:
                htt = htt_tile(tg, hhg)
            htg = htgp.tile([128, 2, 8, 256], BF16, tag="htg", name="htg")
            for hhi in range(8):
                hh = hhg * 8 + hhi
                nc.scalar.activation(
                    out=htg[:, :, hhi, :], in_=htt[:, :, hhi, :],
                    func=AF.Gelu, bias=st["bff1"][:, hh:hh + 1], scale=1.0)
            nxt = (tg, hhg + 1) if hhg < 3 else (tg + 1, 0)
            if nxt[0] < 2 and nxt not in pre["htt"]:
                pre["htt"][nxt] = htt_tile(*nxt)
            for hhi in range(8):
                hh = hhg * 8 + hhi
                w2 = pre["w2"].pop(0) if (tg == 0 and hh < 3) else w2_tile(hh)
                for tt in range(4):
                    pr, ph = tt // 2, tt % 2
                    for half in range(2):
                        nc.tensor.matmul(
                            pss[tt][:, half * 512:(half + 1) * 512],
                            htg[:, pr, hhi, ph * 128:(ph + 1) * 128],
                            w2[:, half * 512:(half + 1) * 512],
                            start=(hh == 0), stop=(hh == 31),
                        )
        for tt in range(4):
            pos = tg * 4 + tt
            yt = yp.tile([128, D], F32, tag="yt")
            nc.vector.tensor_add(out=yt[:], in0=pss[tt][:], in1=x2in[tt][:])
            nc.sync.dma_start(
                y_d.ap()[pos * 128:(pos + 1) * 128, :], yt[:])

    ff2ps.release()
    yp.release()
    x2ip.release()
    htgp.release()
    htp.release()
    w2p.release()
    w1_holder["pb2"].release()
    w1_holder["pb1"].release()
    w1_holder["pa"].release()
    hst_p.release()
    x2t_pool.release()
    p7.release()
    epp.release()
    atp.release()
    attt_pool.release()
    att_pool.release()
    xt2_p.release()
    wqk_p.release()
    pw_p.release()


# ---------------------------------------------------------------------------
# Host wrapper
# ---------------------------------------------------------------------------

_PROG_CACHE = {}


def _get_program(niter=None):
    if niter not in _PROG_CACHE:
        _PROG_CACHE[niter] = build_program(niter)
    return _PROG_CACHE[niter]


def make_in_maps(x, ln1_g, ln1_b, qkv_w, qkv_b, proj_w, proj_b,
                 ln2_g, ln2_b, ff1_w, ff1_b, ff2_w, ff2_b):
    bf = ml_dtypes.bfloat16
    f32 = np.float32

    def pcol(v, n):
        return np.ascontiguousarray(np.asarray(v, f32).reshape(n, 128).T)

    def dimmajor(w, nchunk, ncol):
        return np.ascontiguousarray(
            np.asarray(w, f32).reshape(nchunk, 128, ncol).transpose(1, 0, 2)
        ).astype(bf)

    # fold LN1 gain/bias into the QKV weights and LN2 gain/bias into FF1:
    # LN(x)*g + b feeding W  ==  LN_raw(x) @ (g[:,None]*W) + (b @ W + bias).
    # Device-side LN then omits gain/bias and the transpose copies are pure.
    qkv_w0 = np.asarray(qkv_w, f32)
    g1v, b1v = np.asarray(ln1_g, f32), np.asarray(ln1_b, f32)
    qkv_b = np.asarray(qkv_b, f32) + b1v @ qkv_w0
    qkv_w = g1v[:, None] * qkv_w0
    ff1_w0 = np.asarray(ff1_w, f32)
    g2v, b2v = np.asarray(ln2_g, f32), np.asarray(ln2_b, f32)
    ff1_b = np.asarray(ff1_b, f32) + b2v @ ff1_w0
    ff1_w = g2v[:, None] * ff1_w0
    common = dict(
        wqkv=dimmajor(qkv_w, 8, 3 * D),
        wproj=dimmajor(proj_w, 8, D),
        wff1=dimmajor(ff1_w, 8, HID),
        wff2=dimmajor(ff2_w, 32, D),
        bqk=np.ascontiguousarray(
            np.concatenate([pcol(qkv_b[0:D], 8), pcol(qkv_b[D:2 * D], 8)], axis=1)),
        bv=qkv_b[2 * D:3 * D].copy(),
        bff1=pcol(ff1_b, 32),
        bff2b=np.asarray(ff2_b, f32).astype(bf),
    )
    masks = [np.ascontiguousarray(_masks(p).transpose(1, 0, 2)).astype(bf)
             for p in range(2)]
    perms = [_perm(0), _perm(1)]

    projb = np.asarray(proj_b, f32)
    x = np.asarray(x, f32)
    in_maps = []
    for c in range(N_CORES):
        b, p = c // 2, c % 2
        m = dict(common)
        m["x_full"] = np.ascontiguousarray(x[b]).astype(bf)
        m["x_q"] = np.ascontiguousarray(x[b][perms[p]]).astype(bf)
        m["x_qr"] = (m["x_q"] + projb[None, :]).astype(bf)
        m["mask"] = masks[p]
        in_maps.append(m)
    return in_maps, perms


def kernel(**inputs):
    in_maps, perms = make_in_maps(**{k: np.asarray(v) for k, v in inputs.items()})
    nc = _get_program()
    res = run_bass_kernel_spmd(nc, in_maps, list(range(N_CORES))).results
    y = np.empty((B, T, D), np.float32)
    for c in range(N_CORES):
        b, p = c // 2, c % 2
        y[b][perms[p]] = res[c]["y"]
    return y



# revision 97
# speedup vs baseline: 1.2278x; 1.2112x over previous
"""Trainium2 Bass kernel v3 for a dense transformer block (B=4, T=2048, D=1024, H=16).

Sharding: 8 cores = 4 batches x 2 query-parity groups (zigzag), slots
processed in mixed order ORDER = [7,0,1,6,2,3,5,4] (NK blocks
[2,16,14,4,12,10,6,8]) so that:
  - K/V block-pair pools release progressively (big slots early) while FF1
    state grows -- complementary SBUF footprints;
  - exp (ACT-bound) of big slots overlaps FF1/proj/tails (PE-dense fillers).

Structure per core:
  prefix:   x DMA -> LN1 -> transpose (pipelined per 4-block group; the first
            x tiles lead the weight streams), V/K/Q production interleaved.
  attn:     per position: scores (head-paired K=64, row-tiled) -> exp(ACT) ->
            mask(Pool) -> PV (ones-column denominators) -> scale -> att;
            tail: transpose, proj, + bf16 residual (proj bias folded host-
            side) -> x2, LN2 -> XT2. xq residual tiles prefetched one tail
            ahead. FF1 interleaved as PE filler; gelu -> stage -> HT to DRAM.
  ff2:      streams HT (prefetched one hhg ahead) + W2 (10-deep ring) from
            DRAM, 8-bank PSUM accumulate; ff2 bias folded into x2 reload on
            Pool so the tail is a single DVE add per tile.
Queue isolation (x/xq/weights on ACT, masks on Pool, HT/W2/y on SP) lets the
next iteration's LN1 prefix overlap this iteration's FF2 phase.
"""

import sys

try:
    import concourse  # noqa: F401
except ImportError:
    sys.path.insert(0, "/opt/trn_rl_repo")

import numpy as np
import ml_dtypes

import concourse.bass as bass
import concourse.bacc as bacc
import concourse.tile as tile
from concourse.tile import add_dep_helper
from concourse import mybir
from concourse.bass_utils import run_bass_kernel_spmd

F32 = mybir.dt.float32
BF16 = mybir.dt.bfloat16
AF = mybir.ActivationFunctionType
ALU = mybir.AluOpType
PSUM = bass.MemorySpace.PSUM

B, T, D = 4, 2048, 1024
H, HD = 16, 64
HID = 4 * D
EPS = 1e-5
N_CORES = 8
NB = T // 128  # 16 token blocks per batch
NQ = 8  # query slots per core
TQ = NQ * 128  # query tokens per core

# zigzag query-block assignment (parity balances causal work)
QB = [
    [15, 12, 11, 8, 7, 4, 3, 0],
    [14, 13, 10, 9, 6, 5, 2, 1],
]
NKMAX = [16, 14, 12, 10, 8, 6, 4, 2]
# processing order: position -> zigzag slot index. 2-blk slot first (its Q/K
# deferral fillers run under it), then big slots early so kv pools release
# while FF1 state grows.
ORDER = [7, 0, 1, 6, 2, 3, 5, 4]
NK = [NKMAX[o] for o in ORDER]  # [2, 16, 14, 4, 12, 10, 6, 8]


def _perm(p):
    idx = []
    for pos in range(NQ):
        qb = QB[p][ORDER[pos]]
        idx.extend(range(qb * 128, (qb + 1) * 128))
    return np.array(idx, dtype=np.int64)


def _masks(p):
    """[NQ, 128, 256] multiplicative mask for the last two key blocks per pos."""
    m = np.zeros((NQ, 128, 256), dtype=np.float32)
    tri = np.triu(np.ones((128, 128), np.float32))  # valid where k_local <= q_local
    for pos in range(NQ):
        nk = QB[p][ORDER[pos]] + 1
        nkm = NK[pos]
        for c in range(2):
            ki = nkm - 2 + c
            if ki < nk - 1:
                m[pos, :, c * 128:(c + 1) * 128] = 1.0
            elif ki == nk - 1:
                m[pos, :, c * 128:(c + 1) * 128] = tri
    return m


# ---------------------------------------------------------------------------
# Device program
# ---------------------------------------------------------------------------

def _layernorm_tile(nc, pool, x_ap, out_ap, eps_t, norm_engine=None):
    """LN (without gain/bias) of a [128, D] tile -> out_ap bf16."""
    stats = pool.tile([128, 2, nc.vector.BN_STATS_DIM], F32, tag="ln_stats")
    mv = pool.tile([128, nc.vector.BN_AGGR_DIM], F32, tag="ln_mv")
    nmr = pool.tile([128, 1], F32, tag="ln_nmr")
    xg = x_ap.rearrange("p (s d) -> p s d", s=2)
    for s in range(2):
        nc.vector.bn_stats(out=stats[:, s, :], in_=xg[:, s, :])
    nc.vector.bn_aggr(out=mv[:], in_=stats[:])
    mean = mv[:, 0:1]
    rstd = mv[:, 1:2]
    nc.scalar.activation(out=rstd, in_=rstd, func=AF.Sqrt, bias=eps_t[:], scale=1.0)
    nc.vector.reciprocal(out=rstd, in_=rstd)
    nc.vector.tensor_scalar(
        out=nmr[:], in0=mean, scalar1=rstd, scalar2=-1.0,
        op0=ALU.mult, op1=ALU.mult,
    )
    if norm_engine == "vector":
        nc.vector.tensor_scalar(
            out=out_ap, in0=x_ap, scalar1=rstd, scalar2=nmr[:],
            op0=ALU.mult, op1=ALU.add,
        )
    else:
        nc.scalar.activation(out=out_ap, in_=x_ap, func=AF.Identity,
                             bias=nmr[:], scale=rstd)


def build_program(niter=None, stop_after=None):
    nc = bacc.Bacc("TRN2", target_bir_lowering=False, debug=False,
                   num_devices=N_CORES)

    xf_d = nc.dram_tensor("x_full", [T, D], BF16, kind="ExternalInput")
    xq_d = nc.dram_tensor("x_q", [TQ, D], BF16, kind="ExternalInput")
    xqr_d = nc.dram_tensor("x_qr", [TQ, D], BF16, kind="ExternalInput")
    mask_d = nc.dram_tensor("mask", [128, NQ, 256], BF16, kind="ExternalInput")
    wqkv_d = nc.dram_tensor("wqkv", [128, 8, 3 * D], BF16, kind="ExternalInput")
    wproj_d = nc.dram_tensor("wproj", [128, 8, D], BF16, kind="ExternalInput")
    wff1_d = nc.dram_tensor("wff1", [128, 8, HID], BF16, kind="ExternalInput")
    wff2_d = nc.dram_tensor("wff2", [128, 32, D], BF16, kind="ExternalInput")
    bqk_d = nc.dram_tensor("bqk", [128, 16], F32, kind="ExternalInput")
    bv_d = nc.dram_tensor("bv", [D], F32, kind="ExternalInput")
    bff1_d = nc.dram_tensor("bff1", [128, 32], F32, kind="ExternalInput")
    bff2b_d = nc.dram_tensor("bff2b", [D], BF16, kind="ExternalInput")
    y_d = nc.dram_tensor("y", [TQ, D], F32, kind="ExternalOutput")
    ht_d = nc.dram_tensor("ht_scratch", [128, 4, 32, 256], BF16, kind="Internal")
    x2_d = nc.dram_tensor("x2_scratch", [TQ, D], BF16, kind="Internal")

    def bcast_row(dram_handle, n):
        ap = dram_handle.ap()
        return bass.AP(tensor=ap.tensor, offset=ap.offset, ap=[[0, 128], [1, n]])

    dram = dict(xf=xf_d, xq=xq_d, xqr=xqr_d, wqkv=wqkv_d, wproj=wproj_d,
                wff1=wff1_d, wff2=wff2_d, y=y_d, ht=ht_d, x2=x2_d)

    with tile.TileContext(nc) as tc:
        with tc.tile_pool(name="singles", bufs=1) as singles:
            ident = singles.tile([128, 128], BF16)
            from concourse.masks import make_identity
            make_identity(nc, ident[:])
            nc._ident = ident

            st = {}
            st["eps"] = singles.tile([128, 1], F32, name="eps", tag="eps")
            nc.vector.memset(st["eps"][:], EPS)
            for nm, dt, shape, src in (
                ("bqk", F32, [128, 16], bqk_d), ("bff1", F32, [128, 32], bff1_d),
                ("mask", BF16, [128, NQ, 256], mask_d),
            ):
                st[nm] = singles.tile(shape, dt, name=nm, tag=nm)
                nc.sync.dma_start(st[nm][:], src.ap())
            st["fb2"] = singles.tile([128, D], BF16, name="fb2", tag="fb2")
            nc.gpsimd.dma_start(st["fb2"][:], bcast_row(bff2b_d, D))
            nc._bcast_row = bcast_row
            nc._bv_d = bv_d

            if niter is None:
                _block_body(tc, nc, dram, st, stop_after)
            elif isinstance(niter, str) and niter.startswith("unroll"):
                for _ in range(int(niter[6:])):
                    _block_body(tc, nc, dram, st, stop_after)
            else:
                with tc.For_i(0, niter, 1):
                    _block_body(tc, nc, dram, st, stop_after)

    nc.compile()
    return nc


def _block_body(tc, nc, dram, st, stop_after=None):
    xf_d, xq_d, y_d = dram["xf"], dram["xq"], dram["y"]
    ht_d, x2_d = dram["ht"], dram["x2"]
    eps_t = st["eps"]

    # ---- persistent pools --------------------------------------------------
    # right side (bottom->top): qt, kv[0..7], xtq, xtf
    qt_p = tc.alloc_tile_pool(name="qt", bufs=1, side="right")
    QT = qt_p.tile([128, 8, TQ], BF16, name="QT", tag="qt")
    kv = []
    kvt = []
    for b in range(8):
        p = tc.alloc_tile_pool(name=f"kv{b}", bufs=1, side="right")
        KT2 = p.tile([128, 8, 256], BF16, name=f"KT{b}", tag="kt")
        V2 = p.tile([128, 2, H, HD + 1], BF16, name=f"V{b}", tag="v")
        nc.vector.memset(V2[:, :, :, HD:HD + 1], 1.0)
        kv.append(p)
        kvt.append((KT2, V2))
    xtq_p = tc.alloc_tile_pool(name="xtq", bufs=1, side="right")
    XTQ = xtq_p.tile([128, 8, TQ], BF16, name="XTQ", tag="xtq")
    xtf_p = tc.alloc_tile_pool(name="xtf", bufs=1, side="right")
    XTF = xtf_p.tile([128, 8, T], BF16, name="XTF", tag="xtf")

    def KTb(kk, po, hc):  # [64, 128] stationary view of key block kk
        return kvt[kk // 2][0][po:po + 64, hc, (kk % 2) * 128:(kk % 2 + 1) * 128]

    def Vb(kk, h):  # [128, HD+1] view of key block kk, head h
        return kvt[kk // 2][1][:, kk % 2, h, :]

    pw_p = tc.alloc_tile_pool(name="wproj", bufs=1, side="left")
    PW = pw_p.tile([128, 8, D], BF16, name="PW", tag="pw")

    # ---- prefix: fused LN1 + transpose + V/K/Q production ------------------
    wqk_p = tc.alloc_tile_pool(name="wqk", bufs=3, side="left")
    p1 = tc.alloc_tile_pool(name="p1", bufs=6, side="left")
    p1s = tc.alloc_tile_pool(name="p1s", bufs=8, side="left")
    xln_p = tc.alloc_tile_pool(name="xln", bufs=6, side="left")
    wv_p = tc.alloc_tile_pool(name="wv", bufs=1, side="left")
    pre_tp = tc.alloc_tile_pool(name="pre_tp", bufs=3, space=PSUM)
    pre_mm = tc.alloc_tile_pool(name="pre_mm", bufs=4, space=PSUM)

    def ln_group(x_d, t0, n, XT):
        """LN + transpose n token blocks starting at t0 from x_d into XT."""
        tiles = []
        for u in range(n):
            t = t0 + u
            xt = p1.tile([128, D], BF16, tag="xt", name="xt")
            q = nc.gpsimd if (x_d is xf_d and u % 2 == 0) else nc.scalar
            q.dma_start(xt[:], x_d.ap()[t * 128:(t + 1) * 128, :])
            ln = xln_p.tile([128, D], BF16, tag="ln", name="ln")
            _layernorm_tile(nc, p1s, xt[:], ln[:], eps_t)
            tiles.append(ln)
        for dc in range(8):
            pt = pre_tp.tile([128, n * 128], BF16, tag="tp", name="pt",
                             padded_shape=[128, 1024])
            for u in range(n):
                nc.tensor.transpose(
                    pt[:, u * 128:(u + 1) * 128],
                    tiles[u][:, dc * 128:(dc + 1) * 128], nc._ident[:])
            dst = XT[:, dc, t0 * 128:(t0 + n) * 128]
            if dc % 2 == 1:
                nc.scalar.copy(out=dst, in_=pt[:])
            else:
                nc.vector.tensor_copy(dst, pt[:])

    def emit_v(tg, wv_tiles, vb_t):
        """V for token blocks 4tg..4tg+3 (needs XTF of those blocks)."""
        for vh in range(2):
            for u in range(4):
                t = tg * 4 + u
                ps = pre_mm.tile([128, 512], F32, tag="mm", name="ps")
                for dc in range(8):
                    nc.tensor.matmul(
                        ps[:], XTF[:, dc, t * 128:(t + 1) * 128],
                        wv_tiles[vh][:, dc, :], start=(dc == 0), stop=(dc == 7),
                    )
                nc.vector.tensor_add(
                    out=kvt[t // 2][1][:, t % 2, vh * 8:(vh + 1) * 8, 0:HD],
                    in0=ps[:].rearrange("p (h d) -> p h d", h=8),
                    in1=vb_t[:, vh * 512:(vh + 1) * 512]
                        .rearrange("p (h d) -> p h d", h=8),
                )

    def emit_k(cc, pairs):
        """KT chunk occ=cc-8 for token pair(s) — one weight fetch shared
        across all pairs whose XTF is ready."""
        occ = cc - 8
        w = wqk_p.tile([128, 8, 128], BF16, tag="wqk", name="w")
        q = nc.sync if cc % 2 == 0 else nc.gpsimd
        q.dma_start(
            w[:], dram["wqkv"].ap()[:, :, cc * 128:(cc + 1) * 128])
        for pair in pairs:
            pshs = [pre_mm.tile([128, 512], F32, tag="mm", name="ps")
                    for _ in range(2)]
            for h2 in range(2):
                for dc in range(8):
                    nc.tensor.matmul(
                        pshs[h2][:],
                        w[:, dc, :],
                        XTF[:, dc, (pair * 2 + h2) * 512:
                            (pair * 2 + h2 + 1) * 512],
                        start=(dc == 0), stop=(dc == 7),
                    )
            for g in range(4):
                b = pair * 4 + g
                nc.vector.tensor_scalar_add(
                    out=kvt[b][0][:, occ, :],
                    in0=pshs[g // 2][:, (g % 2) * 256:(g % 2 + 1) * 256],
                    scalar1=st["bqk"][:, cc:cc + 1],
                )

    qw_resident = {}

    def emit_q_prefix(cc):
        """Q chunk cc for positions 0-1 (cols 0-255) only."""
        w = wqk_p.tile([128, 8, 128], BF16, tag="wqk", name="w")
        q = nc.sync if cc % 2 == 0 else nc.gpsimd
        q.dma_start(
            w[:], dram["wqkv"].ap()[:, :, cc * 128:(cc + 1) * 128])
        qw_resident[cc] = w
        ps = pre_mm.tile([128, 256], F32, tag="mm", name="ps",
                         padded_shape=[128, 512])
        for dc in range(8):
            nc.tensor.matmul(
                ps[:], w[:, dc, :], XTQ[:, dc, 0:256],
                start=(dc == 0), stop=(dc == 7),
            )
        nc.vector.tensor_scalar_add(
            out=QT[:, cc, 0:256], in0=ps[:], scalar1=st["bqk"][:, cc:cc + 1],
        )

    # pipelined prefix: x DMAs lead (weight streams issued after the first
    # LN groups so the first tiles aren't queued behind bulk weights)
    wv_tiles = [wv_p.tile([128, 8, 512], BF16, tag=f"wv{vh}", name=f"wv{vh}")
                for vh in range(2)]
    vb_t = wv_p.tile([128, D], F32, name="vb", tag="vb")

    ln_group(xf_d, 0, 2, XTF)
    for vh in range(2):
        nc.sync.dma_start(
            wv_tiles[vh][:], dram["wqkv"].ap()[:, :, 2 * D + vh * 512:
                                               2 * D + (vh + 1) * 512])
    nc.gpsimd.dma_start(vb_t[:], nc._bcast_row(nc._bv_d, D))
    ln_group(xf_d, 2, 2, XTF)
    ln_group(xf_d, 4, 4, XTF)
    emit_v(0, wv_tiles, vb_t)
    ln_group(xf_d, 8, 4, XTF)
    emit_v(1, wv_tiles, vb_t)
    for cc in range(8, 12):
        emit_k(cc, (0,))
    ln_group(xf_d, 12, 4, XTF)
    for cc in range(12, 16):
        emit_k(cc, (0, 1))
    emit_v(2, wv_tiles, vb_t)
    ln_group(xq_d, 0, 4, XTQ)
    emit_v(3, wv_tiles, vb_t)
    for cc in range(8, 12):
        emit_k(cc, (1,))
    ln_group(xq_d, 4, 4, XTQ)
    for cc in range(8):
        emit_q_prefix(cc)

    pre_mm.release()
    pre_tp.release()
    wv_p.release()
    xln_p.release()
    p1s.release()
    p1.release()
    xtf_p.release()
    if stop_after == "prefix":
        xtq_p.release()
        wqk_p.release()
        for b in reversed(range(8)):
            kv[b].release()
        qt_p.release()
        pw_p.release()
        return

    # ---- attention + interleaved FF1 --------------------------------------
    xt2_p = tc.alloc_tile_pool(name="xt2", bufs=1, side="left")
    XT2P = [xt2_p.tile([128, 8, 256], BF16, name=f"XT2_{q}", tag=f"xt2{q}")
            for q in range(4)]

    sc_ps = tc.alloc_tile_pool(name="sc_ps", bufs=2, space=PSUM)
    pv_ps = tc.alloc_tile_pool(name="pv_ps", bufs=1, space=PSUM)
    tp_ps = tc.alloc_tile_pool(name="tp_ps", bufs=1, space=PSUM)
    pj_ps = tc.alloc_tile_pool(name="pj_ps", bufs=1, space=PSUM)
    f1_ps = tc.alloc_tile_pool(name="f1_ps", bufs=1, space=PSUM)

    att_pool = tc.alloc_tile_pool(name="attsl", bufs=2, side="left")
    attt_pool = tc.alloc_tile_pool(name="atttsl", bufs=1, side="left")
    atp = tc.alloc_tile_pool(name="at", bufs=5, side="left")
    epp = tc.alloc_tile_pool(name="ep", bufs=2, side="left")
    p7 = tc.alloc_tile_pool(name="p7", bufs=1, side="left")
    x2t_pool = tc.alloc_tile_pool(name="x2t", bufs=2, side="left")
    hst_p = tc.alloc_tile_pool(name="hstage", bufs=1, side="left")
    w1_holder = {}

    def emit_scores_pair(hc, pos, att_j):
        """Scores+exp+mask for BOTH heads of pair hc at position pos."""
        nblk = NK[pos]
        out = []
        for hp in range(2):
            out.append((2 * hc + hp, pos, [], att_j))
        for g in range((nblk + 7) // 8):
            blo = g * 8
            bhi = min(blo + 8, nblk)
            ncol = (bhi - blo) * 128
            pss = [sc_ps.tile([128, 1024], F32, tag="sc", name="ps")
                   for _ in range(2)]
            for kk in range(blo, bhi):
                for hp in range(2):
                    po = hp * 64
                    nc.tensor.matmul(
                        pss[hp][:, (kk - blo) * 128:(kk - blo + 1) * 128],
                        KTb(kk, po, hc),
                        QT[po:po + 64, hc, pos * 128:(pos + 1) * 128],
                        start=True, stop=True,
                    )
            for hp in range(2):
                at = atp.tile([128, 1024], BF16, tag="at", name="at")
                nc.scalar.activation(out=at[:, 0:ncol], in_=pss[hp][:, 0:ncol],
                                     func=AF.Exp, scale=0.125)
                for kk in range(max(blo, nblk - 2), bhi):
                    mc = (kk - (nblk - 2)) * 128
                    nc.gpsimd.tensor_mul(
                        out=at[:, (kk - blo) * 128:(kk - blo + 1) * 128],
                        in0=at[:, (kk - blo) * 128:(kk - blo + 1) * 128],
                        in1=st["mask"][:, pos, mc:mc + 128],
                    )
                out[hp][2].append((blo, bhi, at))
        return out

    def emit_pv_pair(pend2):
        """PV for both heads of a pair into one [128, 2, HD+1] PSUM tile."""
        pv = pv_ps.tile([128, 2, HD + 1], F32, tag="pv", name="pv")
        for hp, pend in enumerate(pend2):
            h, pos, ats, att_j = pend
            nblk = NK[pos]
            for blo, bhi, at in ats:
                for kk in range(blo, bhi):
                    nc.tensor.matmul(
                        pv[:, hp, :],
                        at[:, (kk - blo) * 128:(kk - blo + 1) * 128],
                        Vb(kk, h),
                        start=(kk == 0), stop=(kk == nblk - 1),
                    )
        for hp, pend in enumerate(pend2):
            h, pos, ats, att_j = pend
            r = epp.tile([128, 1], F32, tag="recip", name="r")
            nc.vector.reciprocal(out=r[:], in_=pv[:, hp, HD:HD + 1])
            nc.vector.tensor_scalar_mul(
                out=att_j[:, h * HD:(h + 1) * HD],
                in0=pv[:, hp, 0:HD], scalar1=r[:],
            )

    xq_pre = {}

    def fetch_xq(pos):
        t = p7.tile([128, D], BF16, tag="xq", name="xq", bufs=2)
        nc.sync.dma_start(t[:], dram["xqr"].ap()[pos * 128:(pos + 1) * 128, :])
        return t

    def slot_tail(pos, att_j):
        """att -> transpose -> proj -> +bias +resid -> x2t; LN2 -> XT2; spill."""
        attt = attt_pool.tile([128, 8, 128], BF16, tag="attt", name="attt")
        for g2 in range(2):
            pt = tp_ps.tile([128, 512], BF16, tag="tp", name="pt",
                            padded_shape=[128, 1024])
            for u in range(4):
                dc = g2 * 4 + u
                nc.tensor.transpose(
                    pt[:, u * 128:(u + 1) * 128],
                    att_j[:, dc * 128:(dc + 1) * 128], nc._ident[:])
            nc.vector.tensor_copy(attt[:, g2 * 4:(g2 + 1) * 4, :], pt[:])
        xq = xq_pre.pop(pos, None)
        if xq is None:
            xq = fetch_xq(pos)
        if pos + 1 < NQ and pos + 1 not in xq_pre:
            xq_pre[pos + 1] = fetch_xq(pos + 1)
        x2t = x2t_pool.tile([128, D], BF16, tag="x2t", name="x2t")
        for half in range(2):
            ps = pj_ps.tile([128, 512], F32, tag="pj", name="ps")
            for hcc in range(8):
                nc.tensor.matmul(
                    ps[:],
                    attt[:, hcc, :],
                    PW[:, hcc, half * 512:(half + 1) * 512],
                    start=(hcc == 0), stop=(hcc == 7),
                )
            nc.vector.tensor_add(
                out=x2t[:, half * 512:(half + 1) * 512],
                in0=ps[:], in1=xq[:, half * 512:(half + 1) * 512])
        x2_out_insts[pos] = nc.sync.dma_start(
            x2_d.ap()[pos * 128:(pos + 1) * 128, :], x2t[:])
        # LN2 on x2t -> bf16, then transpose into the position's XT2 pair tile
        ln2 = p7.tile([128, D], BF16, tag="ln2", name="ln2")
        _layernorm_tile(nc, epp, x2t[:], ln2[:], eps_t, norm_engine="vector")
        pr, qcol = pos // 2, (pos % 2) * 128
        for g2 in range(2):
            pt = tp_ps.tile([128, 512], BF16, tag="tp", name="pt",
                            padded_shape=[128, 1024])
            for u in range(4):
                dc = g2 * 4 + u
                nc.tensor.transpose(
                    pt[:, u * 128:(u + 1) * 128],
                    ln2[:, dc * 128:(dc + 1) * 128], nc._ident[:])
            nc.vector.tensor_copy(
                XT2P[pr][:, g2 * 4:(g2 + 1) * 4, qcol:qcol + 128],
                pt[:].rearrange("p (a b) -> p a b", a=4))

    def w_fetch(cc):
        w = wqk_p.tile([128, 8, 128], BF16, tag="wqk", name="w")
        nc.sync.dma_start(
            w[:], dram["wqkv"].ap()[:, :, cc * 128:(cc + 1) * 128])
        return w

    def q_filler(cc, w):
        """Deferred Q production: chunk cc, column blocks 1-3 (768 cols)."""
        for cb in range(1, 4):
            ps = f1_ps.tile([128, 2, 256], F32, tag="f1", name="ps")
            for dc in range(8):
                nc.tensor.matmul(
                    ps[:, 0, :], w[:, dc, :],
                    XTQ[:, dc, cb * 256:(cb + 1) * 256],
                    start=(dc == 0), stop=(dc == 7),
                )
            nc.vector.tensor_scalar_add(
                out=QT[:, cc, cb * 256:(cb + 1) * 256], in0=ps[:, 0, :],
                scalar1=st["bqk"][:, cc:cc + 1],
            )

    def ff1_chunk(pair, hhg):
        """FF1 hidden group hhg (8 hh) for position pair (256 tokens)."""
        if hhg < 2:
            W1, hoff = w1_holder["a"], 0
        elif hhg == 2:
            W1, hoff = w1_holder["b1"], 16
        else:
            W1, hoff = w1_holder["b2"], 24
        stage = hst_p.tile([128, 8, 256], BF16, tag="hs", name="hs")
        for hhi in range(0, 8, 2):
            if pair == 3:
                # post-attention: pv/tp/pj banks are dead — cycle all four
                # 1-bank pools so the MMs never stall on the stage copies
                pool, tag = ((f1_ps, "f1"), (pv_ps, "pv"),
                             (tp_ps, "tp"), (pj_ps, "pj"))[hhi // 2]
                ps = pool.tile([128, 2, 256], F32, tag=tag, name="ps")
            else:
                ps = f1_ps.tile([128, 2, 256], F32, tag="f1", name="ps")
            for c in range(2):
                hh = hhg * 8 + hhi + c
                for dc in range(8):
                    nc.tensor.matmul(
                        ps[:, c, :],
                        W1[:, dc, (hh - hoff) * 128:(hh - hoff + 1) * 128],
                        XT2P[pair][:, dc, :],
                        start=(dc == 0), stop=(dc == 7),
                    )
            if hhi % 4 == 0 or pair == 3:
                nc.vector.tensor_copy(stage[:, hhi:hhi + 2, :], ps[:])
            else:
                nc.scalar.copy(out=stage[:, hhi:hhi + 2, :], in_=ps[:])
        ht_out_insts[(pair, hhg)] = nc.sync.dma_start(
            ht_d.ap()[:, pair, hhg * 8:(hhg + 1) * 8, :], stage[:])

    ht_out_insts = {}
    x2_out_insts = {}

    # filler queue: (weight_cc_or_None, compute) pairs, ~3-7us of PE work
    # each; weight DMAs prefetched 3 fillers ahead via the wqk ring.
    fillers = []
    # chunks 5-7 are still resident in the 3-deep wqk ring from the prefix
    # Q pass: run those fillers first with no refetch, giving the cc0-4
    # refetch DMAs a head start over the pos0 drain.
    for i in (5, 6, 7):
        fillers.append((None, lambda w, c=i: q_filler(c, qw_resident[c])))
    for i in range(5):
        fillers.append((i, lambda w, c=i: q_filler(c, w)))
    prefetched = []

    def _prime():
        while len(prefetched) < 3 and fillers:
            cc2, fn2 = fillers.pop(0)
            prefetched.append((fn2, w_fetch(cc2) if cc2 is not None else None))

    _prime()

    def drain_filler(k=1):
        for _ in range(k):
            _prime()
            if not prefetched:
                return
            fn, w = prefetched.pop(0)
            fn(w)

    pending = None
    done = []
    wload_sched = {}
    for qtr in range(4):
        wload_sched[(0, qtr)] = (
            lambda q=qtr: nc.sync.dma_start(
                PW[:, 2 * q:2 * q + 2, :],
                dram["wproj"].ap()[:, 2 * q:2 * q + 2, :]))
    for pos in range(NQ):
        att_j = att_pool.tile([128, D], BF16, tag="att", name="att_j")
        for hc in range(8):
            if (pos, hc) in wload_sched:
                wload_sched.pop((pos, hc))()
            cur2 = emit_scores_pair(hc, pos, att_j)
            if pending is not None:
                emit_pv_pair(pending)
            pending = cur2
            if hc == 0 and done:
                slot_tail(*done.pop())
            elif pos < 2:
                drain_filler(2)
            elif pos < 4:
                if hc in (2, 4, 6):
                    drain_filler(2 if len(fillers) + len(prefetched) > 1 else 1)
            else:
                drain_filler(1)
        done.append((pos, att_j))
        if pos == 0:
            assert not fillers and not prefetched  # deferred Q done
            xtq_p.release()
            # W1 piece A (hh 0-15): allocate now (reusing XTQ's bytes) but
            # stream the halves mid-pos1 on sync, clear of the pos0 tail's
            # xq/x2 DMAs and of Pool's mask path.
            w1a_p = tc.alloc_tile_pool(name="w1a", bufs=1, side="left")
            W1A = w1a_p.tile([128, 8, HID // 2], BF16, name="W1A", tag="w1a")
            wload_sched[(1, 2)] = lambda: nc.sync.dma_start(
                W1A[:, :, 0:HID // 4], dram["wff1"].ap()[:, :, 0:HID // 4])
            wload_sched[(1, 5)] = lambda: nc.sync.dma_start(
                W1A[:, :, HID // 4:HID // 2],
                dram["wff1"].ap()[:, :, HID // 4:HID // 2])
            w1_holder["a"] = W1A
            w1_holder["pa"] = w1a_p
        if pos == 1:
            kv[7].release()
            # W1 piece B1 (hh 16-23), streamed mid-pos2
            w1b1_p = tc.alloc_tile_pool(name="w1b1", bufs=1, side="left")
            W1B1 = w1b1_p.tile([128, 8, HID // 4], BF16, name="W1B1",
                               tag="w1b1")
            wload_sched[(2, 3)] = lambda: nc.sync.dma_start(
                W1B1[:], dram["wff1"].ap()[:, :, HID // 2:3 * HID // 4])
            w1_holder["b1"] = W1B1
            w1_holder["pb1"] = w1b1_p
        if pos == 2:
            kv[6].release()
            # W1 piece B2 (hh 24-31), streamed mid-pos3
            w1b2_p = tc.alloc_tile_pool(name="w1b2", bufs=1, side="left")
            W1B2 = w1b2_p.tile([128, 8, HID // 4], BF16, name="W1B2",
                               tag="w1b2")
            wload_sched[(3, 3)] = lambda: nc.sync.dma_start(
                W1B2[:], dram["wff1"].ap()[:, :, 3 * HID // 4:])
            w1_holder["b2"] = W1B2
            w1_holder["pb2"] = w1b2_p
            for hhg in (0, 1):
                fillers.append((None, lambda w, h_=hhg: ff1_chunk(0, h_)))
        if pos == 3:
            for hhg in (2, 3):
                fillers.append((None, lambda w, h_=hhg: ff1_chunk(0, h_)))
        if pos == 4:
            for hhg in range(4):
                fillers.append((None, lambda w, h_=hhg: ff1_chunk(1, h_)))
        if pos == 6:
            for hhg in range(4):
                fillers.append((None, lambda w, h_=hhg: ff1_chunk(2, h_)))
    emit_pv_pair(pending)
    slot_tail(*done.pop())
    while fillers or prefetched:
        drain_filler()

    for b in (5, 4, 3, 2, 1, 0):
        kv[b].release()
    qt_p.release()

    # FF2 pools + tg0 prefetch issued BEFORE the last FF1 pair so the sync
    # queue has tg0's streams in flight while pair-3 FF1 finishes on PE.
    w2p = tc.alloc_tile_pool(name="w2", bufs=10, side="left")
    htp = tc.alloc_tile_pool(name="htin", bufs=2, side="left")
    htgp = tc.alloc_tile_pool(name="htgel", bufs=2, side="left")
    x2ip = tc.alloc_tile_pool(name="x2in", bufs=4, side="left")
    yp = tc.alloc_tile_pool(name="yp", bufs=2, side="left")

    pre = {}

    def x2_tile(pos):
        x2i = x2ip.tile([128, D], BF16, tag="x2i", name="x2i")
        rd = nc.sync.dma_start(x2i[:], x2_d.ap()[pos * 128:(pos + 1) * 128, :])
        add_dep_helper(rd.ins, x2_out_insts[pos].ins, True,
                       "x2 scratch RAW across DMA queues")
        # fold the ff2 bias in on Pool (idle during FF2) so the tail is one add
        nc.gpsimd.tensor_add(out=x2i[:], in0=x2i[:], in1=st["fb2"][:])
        return x2i

    def htt_tile(tg, hhg):
        htt = htp.tile([128, 2, 8, 256], BF16, tag="ht", name="htt")
        rd = nc.sync.dma_start(
            htt[:], ht_d.ap()[:, 2 * tg:2 * tg + 2, hhg * 8:(hhg + 1) * 8, :])
        for pr in (2 * tg, 2 * tg + 1):
            add_dep_helper(rd.ins, ht_out_insts[(pr, hhg)].ins, True,
                           "ht scratch RAW across DMA queues")
        return htt

    def w2_tile(hh):
        w2 = w2p.tile([128, D], BF16, tag="w2")
        nc.sync.dma_start(w2[:], dram["wff2"].ap()[:, hh, :])
        return w2

    pre["x2"] = [x2_tile(pos) for pos in range(4)]
    pre["htt"] = {(0, 0): htt_tile(0, 0)}
    pre["w2"] = [w2_tile(hh) for hh in range(3)]

    for hhg in range(4):
        ff1_chunk(3, hhg)

    f1_ps.release()
    pj_ps.release()
    tp_ps.release()
    pv_ps.release()
    sc_ps.release()
    ff2ps = tc.alloc_tile_pool(name="ff2ps", bufs=4, space=PSUM)

    for tg in range(2):
        x2in = pre.pop("x2") if tg == 0 else [x2_tile(tg * 4 + tt)
                                              for tt in range(4)]
        pss = [ff2ps.tile([128, 1024], F32, name="ym", tag="ym")
               for _ in range(4)]
        for hhg in range(4):
            htt = pre["htt"].pop((tg, hhg), None)
            if htt is None:
                htt = htt_tile(tg, hhg)
            htg = htgp.tile([128, 2, 8, 256], BF16, tag="htg", name="htg")
            for hhi in range(8):
                hh = hhg * 8 + hhi
                nc.scalar.activation(
                    out=htg[:, :, hhi, :], in_=htt[:, :, hhi, :],
                    func=AF.Gelu, bias=st["bff1"][:, hh:hh + 1], scale=1.0)
            nxt = (tg, hhg + 1) if hhg < 3 else (tg + 1, 0)
            if nxt[0] < 2 and nxt not in pre["htt"]:
                pre["htt"][nxt] = htt_tile(*nxt)
            for hhi in range(8):
                hh = hhg * 8 + hhi
                w2 = pre["w2"].pop(0) if (tg == 0 and hh < 3) else w2_tile(hh)
                for tt in range(4):
                    pr, ph = tt // 2, tt % 2
                    for half in range(2):
                        nc.tensor.matmul(
                            pss[tt][:, half * 512:(half + 1) * 512],
                            htg[:, pr, hhi, ph * 128:(ph + 1) * 128],
                            w2[:, half * 512:(half + 1) * 512],
                            start=(hh == 0), stop=(hh == 31),
                        )
        for tt in range(4):
            pos = tg * 4 + tt
            yt = yp.tile([128, D], F32, tag="yt")
            nc.vector.tensor_add(out=yt[:], in0=pss[tt][:], in1=x2in[tt][:])
            nc.sync.dma_start(
                y_d.ap()[pos * 128:(pos + 1) * 128, :], yt[:])

    ff2ps.release()
    yp.release()
    x2ip.release()
    htgp.release()
    htp.release()
    w2p.release()
    w1_holder["pb2"].release()
    w1_holder["pb1"].release()
    w1_holder["pa"].release()
    hst_p.release()
    x2t_pool.release()
    p7.release()
    epp.release()
    atp.release()
    attt_pool.release()
    att_pool.release()
    xt2_p.release()
    wqk_p.release()
    pw_p.release()


# ---------------------------------------------------------------------------
# Host wrapper
# ---------------------------------------------------------------------------

_PROG_CACHE = {}


def _get_program(niter=None):
    if niter not in _PROG_CACHE:
        _PROG_CACHE[niter] = build_program(niter)
    return _PROG_CACHE[niter]


def make_in_maps(x, ln1_g, ln1_b, qkv_w, qkv_b, proj_w, proj_b,
                 ln2_g, ln2_b, ff1_w, ff1_b, ff2_w, ff2_b):
    bf = ml_dtypes.bfloat16
    f32 = np.float32

    def pcol(v, n):
        return np.ascontiguousarray(np.asarray(v, f32).reshape(n, 128).T)

    def dimmajor(w, nchunk, ncol):
        return np.ascontiguousarray(
            np.asarray(w, f32).reshape(nchunk, 128, ncol).transpose(1, 0, 2)
        ).astype(bf)

    # fold LN1 gain/bias into the QKV weights and LN2 gain/bias into FF1:
    # LN(x)*g + b feeding W  ==  LN_raw(x) @ (g[:,None]*W) + (b @ W + bias).
    # Device-side LN then omits gain/bias and the transpose copies are pure.
    qkv_w0 = np.asarray(qkv_w, f32)
    g1v, b1v = np.asarray(ln1_g, f32), np.asarray(ln1_b, f32)
    qkv_b = np.asarray(qkv_b, f32) + b1v @ qkv_w0
    qkv_w = g1v[:, None] * qkv_w0
    ff1_w0 = np.asarray(ff1_w, f32)
    g2v, b2v = np.asarray(ln2_g, f32), np.asarray(ln2_b, f32)
    ff1_b = np.asarray(ff1_b, f32) + b2v @ ff1_w0
    ff1_w = g2v[:, None] * ff1_w0
    common = dict(
        wqkv=dimmajor(qkv_w, 8, 3 * D),
        wproj=dimmajor(proj_w, 8, D),
        wff1=dimmajor(ff1_w, 8, HID),
        wff2=dimmajor(ff2_w, 32, D),
        bqk=np.ascontiguousarray(
            np.concatenate([pcol(qkv_b[0:D], 8), pcol(qkv_b[D:2 * D], 8)], axis=1)),
        bv=qkv_b[2 * D:3 * D].copy(),
        bff1=pcol(ff1_b, 32),
        bff2b=np.asarray(ff2_b, f32).astype(bf),
    )
    masks = [np.ascontiguousarray(_masks(p).transpose(1, 0, 2)).astype(bf)
             for p in range(2)]
    perms = [_perm(0), _perm(1)]

    projb = np.asarray(proj_b, f32)
    x = np.asarray(x, f32)
    in_maps = []
    for c in range(N_CORES):
        b, p = c // 2, c % 2
        m = dict(common)
        m["x_full"] = np.ascontiguousarray(x[b]).astype(bf)
        m["x_q"] = np.ascontiguousarray(x[b][perms[p]]).astype(bf)
        m["x_qr"] = (m["x_q"] + projb[None, :]).astype(bf)
        m["mask"] = masks[p]
        in_maps.append(m)
    return in_maps, perms


def kernel(**inputs):
    in_maps, perms = make_in_maps(**{k: np.asarray(v) for k, v in inputs.items()})
    nc = _get_program()
    res = run_bass_kernel_spmd(nc, in_maps, list(range(N_CORES))).results
    y = np.empty((B, T, D), np.float32)
    for c in range(N_CORES):
        b, p = c // 2, c % 2
        y[b][perms[p]] = res[c]["y"]
    return y

